# revision 1
# baseline (speedup 1.0000x reference)
"""Decoder layer (self-attn + cross-attn + FFN, 3 post-LNs) on 8 TRN2 cores.

Sharding: core c -> (batch b = c//4, query block q = c%4 of 512 tokens).
Uniform SPMD program; causality via per-core data (permuted key order puts the
own block first, so the diagonal is always key-tiles [0, S/128) and
fully-masked keys get an exp bias of -30 from a per-core bias column).
All matmuls run as float32r (fp32 storage, ~tf32 precision, bf16 speed).
"""
import sys, os, uuid, hashlib, types, glob
sys.path.insert(0, '/opt/trn_rl_repo')
import numpy as np

import concourse.bass as bass
import concourse.bacc as bacc
import concourse.mybir as mybir
import concourse.tile as tile

F32 = mybir.dt.float32
F32R = mybir.dt.float32r
AF = mybir.ActivationFunctionType
ALU = mybir.AluOpType

NEFF_CACHE = os.environ.get("MY_NEFF_CACHE", "/root/.my_neff_cache")


# ---------------------------------------------------------------- program ---
def build_program(D=1024, H=16, FF=4096, S=512, E=2048, n_cores=8):
    HD = 64
    assert D == H * HD
    KD = D // 128          # contraction tiles over model dim
    NP = H // 2            # head pairs (=feature 128-tiles of q/k)
    TT = S // 128          # token tiles of own block
    KT = E // 128          # key tiles
    FT = FF // 128
    DIAG = S // 128        # first DIAG key tiles are the causal diagonal
    VW = H * 128           # V_ext width
    CH = min(512, D)       # free-dim chunk over model features
    NOF = D // CH
    KC = min(512, E)       # key chunk
    TC = min(512, E)       # token chunk for projections
    NTC = E // TC

    nc = bacc.Bacc("TRN2", target_bir_lowering=False, debug=False,
                   num_devices=n_cores)

    def din(name, shape, dt=F32R):
        return nc.dram_tensor(name, shape, dt, kind="ExternalInput")

    yT_d = din("yT", [D, E])
    yblk_d = din("yblk", [S, D], F32)
    xT_d = din("xT", [D, E])
    kbias_d = din("kbias", [128, KT], F32)
    ident_d = din("ident", [128, 128])
    ones128_d = din("ones128", [1, 128])
    ozsb_d = din("ozsb", [128, H * 64])
    wq_d, wk_d, wv_d = din("wq", [D, D]), din("wk", [D, D]), din("wv", [D, D])
    wsa_d = din("wsa", [D, D])
    wq2_d, wk2_d, wv2_d = din("wq2", [D, D]), din("wk2", [D, D]), din("wv2", [D, D])
    wca_d = din("wca", [D, D])
    wff1_d, wff2_d = din("wff1", [D, FF]), din("wff2", [FF, D])
    bqT_d, bkT_d = din("bqT", [128, NP], F32), din("bkT", [128, NP], F32)
    bq2T_d, bk2T_d = din("bq2T", [128, NP], F32), din("bk2T", [128, NP], F32)
    bff1T_d = din("bff1T", [128, FT], F32)
    bv_d, bv2_d = din("bv_r", [1, D]), din("bv2_r", [1, D])
    bsa_d, bca_d, bff2_d = din("bsa_r", [1, D]), din("bca_r", [1, D]), din("bff2_r", [1, D])
    gb_d = {n: din(n, [1, D], F32) for n in ("g1", "b1", "g2", "b2", "g3", "b3")}

    out_d = nc.dram_tensor("out_blk", [S, D], F32, kind="ExternalOutput")

    kfm_s = nc.dram_tensor("kfm_s", [D, E], F32R)
    vext_s = nc.dram_tensor("vext_s", [128, (E // 128) * VW], F32R)
    k2fm_s = nc.dram_tensor("k2fm_s", [D, E], F32R)
    v2ext_s = nc.dram_tensor("v2ext_s", [128, (E // 128) * VW], F32R)

    with tile.TileContext(nc) as tc:
        cpool = tc.alloc_tile_pool(name="const", bufs=1)
        ident = cpool.tile([128, 128], F32R, tag="ident", name="ident")
        ones128 = cpool.tile([1, 128], F32R, tag="ones128", name="ones128")
        ozsb = cpool.tile([128, H * 64], F32R, tag="ozsb", name="ozsb")
        kbias = cpool.tile([128, KT], F32, tag="kbias", name="kbias")
        eps_t = cpool.tile([128, 1], F32, tag="eps", name="eps")
        nc.vector.memset(eps_t[:], 1e-9)
        for t, d in [(ident, ident_d), (ones128, ones128_d), (ozsb, ozsb_d),
                     (kbias, kbias_d)]:
            nc.sync.dma_start(t[:], d[:])
        bqT = cpool.tile([128, NP], F32, tag="bqT", name="bqT")
        bkT = cpool.tile([128, NP], F32, tag="bkT", name="bkT")
        bq2T = cpool.tile([128, NP], F32, tag="bq2T", name="bq2T")
        bk2T = cpool.tile([128, NP], F32, tag="bk2T", name="bk2T")
        bff1T = cpool.tile([128, FT], F32, tag="bff1T", name="bff1T")
        for t, d in [(bqT, bqT_d), (bkT, bkT_d), (bq2T, bq2T_d),
                     (bk2T, bk2T_d), (bff1T, bff1T_d)]:
            nc.sync.dma_start(t[:], d[:])
        p_att = tc.alloc_tile_pool(name="p_att", bufs=1)
        qfm = [p_att.tile([128, S], F32R, tag=f"qfm{p}", name=f"qfm{p}") for p in range(NP)]
        avfm = [p_att.tile([128, S], F32R, tag=f"avfm{p}", name=f"avfm{p}") for p in range(NP)]
        y1 = [p_att.tile([128, D], F32, tag=f"y1_{t}", name=f"y1_{t}") for t in range(TT)]

        # ------------- chunked K/V/Q projections (src streamed) ------------
        def kvq_proj(*a, **kw):
            for _ in kvq_gen(*a, **kw):
                pass

        def kvq_gen(srcT_d, wq_, wk_, wv_, bq_, bk_, bv_, q_dst, kfm_dst,
                    vext_dst, with_q, src_bufs=2, ps_bufs=2, lean_v=False):
            with (
                tc.tile_pool(name="pj_src", bufs=src_bufs) as srp,
                tc.tile_pool(name="pj_wk", bufs=1) as wkp,
                tc.tile_pool(name="pj_wv", bufs=2) as wvp,
                tc.tile_pool(name="pj_ev", bufs=2) as evp,
                tc.tile_pool(name="pj_ps", bufs=ps_bufs, space="PSUM") as pp,
            ):
                # ones/zeros filler into vext scratch
                for tb in range(KT):
                    nc.gpsimd.dma_start(
                        vext_dst[:, tb * VW:(tb + 1) * VW]
                        .rearrange("p (h c) -> p h c", c=128)[:, :, 64:128],
                        ozsb[:].rearrange("p (h c) -> p h c", c=64))
                bvrow = evp.tile([1, D], F32R, tag="bvrow", name="bvrow")
                nc.sync.dma_start(bvrow[:], bv_[:])
                wk_t = []
                wv_t = []
                for k in range(KD):
                    t = wkp.tile([128, D], F32R, tag=f"wk{k}", name=f"wk{k}")
                    nc.sync.dma_start(t[:], wk_[k * 128:(k + 1) * 128, :])
                    wk_t.append(t)
                    t2 = wkp.tile([128, D], F32R, tag=f"wv{k}", name=f"wv{k}")
                    nc.sync.dma_start(t2[:], wv_[k * 128:(k + 1) * 128, :])
                    wv_t.append(t2)
                yield
                for tch in range(NTC):
                    src = []
                    for k in range(KD):
                        t = srp.tile([128, TC], F32R, tag=f"src{k}", name=f"src{k}")
                        nc.sync.dma_start(
                            t[:], srcT_d[k * 128:(k + 1) * 128,
                                         tch * TC:(tch + 1) * TC])
                        src.append(t)
                    # K projection, feature-major out
                    for p in range(NP):
                        ps = pp.tile([128, TC], F32, tag="psk", name="psk")
                        for k in range(KD):
                            nc.tensor.matmul(
                                ps[:], wk_t[k][:, p * 128:(p + 1) * 128],
                                src[k][:], start=(k == 0), stop=(k == KD - 1))
                        ev = evp.tile([128, TC], F32R, tag="kev", name="kev")
                        nc.vector.tensor_scalar_add(ev[:], ps[:],
                                                    bk_[:, p:p + 1])
                        nc.gpsimd.dma_start(
                            kfm_dst[p * 128:(p + 1) * 128,
                                    tch * TC:(tch + 1) * TC], ev[:])
                    # Q projection (own block = chunk 0, cols 0:S)
                    if with_q and tch == 0:
                        for p in range(NP):
                            ps = pp.tile([128, S], F32, tag="psq", name="psq")
                            for k in range(KD):
                                wqt = wvp.tile([128, 128], F32R, tag="wqs", name="wqs")
                                nc.sync.dma_start(
                                    wqt[:], wq_[k * 128:(k + 1) * 128,
                                                p * 128:(p + 1) * 128])
                                nc.tensor.matmul(
                                    ps[:], wqt[:],
                                    src[k][:, 0:S], start=(k == 0),
                                    stop=(k == KD - 1))
                            nc.vector.tensor_scalar_add(q_dst[p][:], ps[:],
                                                        bq_[:, p:p + 1])
                    # V projection, token-major out into vext scratch
                    for tt_ in range(TC // 128):
                        gtok = tch * TC + tt_ * 128
                        if lean_v:
                            psv = []
                            for vf in range(NOF):
                                pv = pp.tile([128, CH], F32, tag="psv",
                                             name="psv")
                                for k in range(KD):
                                    nc.tensor.matmul(
                                        pv[:],
                                        src[k][:, tt_ * 128:(tt_ + 1) * 128],
                                        wv_t[k][:, vf * CH:(vf + 1) * CH],
                                        start=(k == 0), stop=False)
                                psv.append(pv)
                        else:
                            psv = [pp.tile([128, CH], F32, tag=f"psv{vf}",
                                           name=f"psv{vf}") for vf in range(NOF)]
                            for k in range(KD):
                                for vf in range(NOF):
                                    nc.tensor.matmul(
                                        psv[vf][:],
                                        src[k][:, tt_ * 128:(tt_ + 1) * 128],
                                        wv_t[k][:, vf * CH:(vf + 1) * CH],
                                        start=(k == 0), stop=False)
                        for vf in range(NOF):
                            nc.tensor.matmul(psv[vf][:], ones128[:],
                                             bvrow[:, vf * CH:(vf + 1) * CH],
                                             start=False, stop=True)
                            ev = evp.tile([128, CH], F32R, tag="vev", name="vev")
                            nc.vector.tensor_copy(ev[:], psv[vf][:])
                            nhd = CH // 64
                            kt_ = gtok // 128
                            nc.gpsimd.dma_start(
                                vext_dst[:, kt_ * VW:(kt_ + 1) * VW]
                                .rearrange("p (h c) -> p h c", c=128)
                                [:, vf * nhd:(vf + 1) * nhd, 0:64],
                                ev[:].rearrange("p (h c) -> p h c", c=64))
                    yield

        kvq_proj(yT_d, wq_d, wk_d, wv_d, bqT, bkT, bv_d, qfm, kfm_s,
                 vext_s, with_q=True)

        # ------------------------- attention -------------------------------
        def attention(*a, **kw):
            for _ in att_gen(*a, **kw):
                pass

        def att_gen(q_tiles, kfm_src, vext_src, out_tiles, causal,
                    ss_bufs=2, pav_bufs=2):
            with (
                tc.tile_pool(name="at_sb", bufs=3) as sp,
                tc.tile_pool(name="at_ps", bufs=ss_bufs, space="PSUM") as pp,
                tc.tile_pool(name="at_pav", bufs=pav_bufs, space="PSUM") as pav,
                tc.tile_pool(name="at_n", bufs=2) as npool,
            ):
                for p in range(NP):
                    psA = pav.tile([128, S], F32, tag="psavA", name="psavA")
                    psB = pav.tile([128, S], F32, tag="psavB", name="psavB")
                    nkj = KC // 128
                    for ktc in range(E // KC):
                        kl = sp.tile([128, KC], F32R, tag="kl", name="kl")
                        nc.scalar.dma_start(
                            kl[:], kfm_src[p * 128:(p + 1) * 128,
                                           ktc * KC:(ktc + 1) * KC])
                        vl4 = sp.tile([128, nkj, 256], F32R, tag="vl", name="vl")
                        nc.scalar.dma_start(
                            vl4[:], vext_src[:, ktc * nkj * VW:(ktc + 1) * nkj * VW]
                            .rearrange("p (t w) -> p t w", w=VW)
                            [:, :, p * 256:(p + 1) * 256])
                        for kj in range(nkj):
                            kt = ktc * nkj + kj
                            vl = vl4[:, kj]
                            ss = pp.tile([128, 2 * S], F32, tag="ss", name="ss")
                            nc.tensor.matmul(
                                ss[:, 0:S], kl[0:64, kj * 128:(kj + 1) * 128],
                                q_tiles[p][0:64, :], start=True, stop=True,
                                tile_position=(0, 0))
                            nc.tensor.matmul(
                                ss[:, S:2 * S],
                                kl[64:128, kj * 128:(kj + 1) * 128],
                                q_tiles[p][64:128, :], start=True, stop=True,
                                tile_position=(64, 0))
                            pr2 = sp.tile([128, 2 * S], F32R, tag="pr2", name="pr2")
                            bias_ap = kbias[:, kt:kt + 1] if causal else 0.0
                            nc.scalar.activation(pr2[:], ss[:], AF.Exp,
                                                 bias=bias_ap, scale=0.125)
                            if causal and kt < DIAG:
                                for half in range(2):
                                    nc.gpsimd.affine_select(
                                        out=pr2[:, half * S:(half + 1) * S],
                                        in_=pr2[:, half * S:(half + 1) * S],
                                        compare_op=ALU.is_ge, fill=0.0,
                                        base=-kt * 128, channel_multiplier=-1,
                                        pattern=[[1, S]])
                            nc.tensor.matmul(psA[:], vl[:, 0:128],
                                             pr2[:, 0:S], start=(kt == 0),
                                             stop=(kt == KT - 1))
                            nc.tensor.matmul(psB[:], vl[:, 128:256],
                                             pr2[:, S:2 * S], start=(kt == 0),
                                             stop=(kt == KT - 1))
                    recA = npool.tile([1, S], F32, tag="recA", name="recA")
                    recB = npool.tile([1, S], F32, tag="recB", name="recB")
                    nc.vector.reciprocal(recA[:], psA[64:65, :])
                    nc.vector.reciprocal(recB[:], psB[64:65, :])
                    rbA = npool.tile([64, S], F32, tag="rbA", name="rbA")
                    rbB = npool.tile([64, S], F32, tag="rbB", name="rbB")
                    nc.gpsimd.partition_broadcast(rbA[:], recA[:], channels=64)
                    nc.gpsimd.partition_broadcast(rbB[:], recB[:], channels=64)
                    avB = npool.tile([64, S], F32R, tag="avB", name="avB")
                    nc.vector.tensor_mul(out_tiles[p][0:64, :], psA[0:64, :],
                                         rbA[:])
                    nc.vector.tensor_mul(avB[:], psB[0:64, :], rbB[:])
                    nc.sync.dma_start(out_tiles[p][64:128, :], avB[:])
                    yield
                yield  # hold pools open until the driver drains us last

        # Interleave self-attention (ACT-heavy) with cross K/V projection
        # (PE-heavy, independent) so the PE fills attention's exp-wait gaps.
        g_att = att_gen(qfm, kfm_s, vext_s, avfm, causal=True,
                        ss_bufs=2, pav_bufs=1)
        g_cross = kvq_gen(xT_d, None, wk2_d, wv2_d, None, bk2T, bv2_d, None,
                          k2fm_s, v2ext_s, with_q=False, src_bufs=1, ps_bufs=1,
                          lean_v=True)
        att_done = False
        next(g_att)             # opens att pools first (released last)
        while True:
            try:
                next(g_cross)
            except StopIteration:
                break
            if not att_done:
                try:
                    next(g_att)
                except StopIteration:
                    att_done = True
        if not att_done:
            for _ in g_att:
                pass

        # ------------------ fc + residual + LN (token-major) ---------------
        def fc_ln(act_tiles, w_d_, brow_d_, resid, g_d_, b_d_, out_tiles, nk,
                  resid_is_dram=False):
            with (
                tc.tile_pool(name="fc_w", bufs=3) as wp,
                tc.tile_pool(name="fc_ps", bufs=1, space="PSUM") as pp,
                tc.tile_pool(name="fc_sb", bufs=1) as sp,
            ):
                brow_ = sp.tile([1, D], F32R, tag="brow", name="brow")
                nc.sync.dma_start(brow_[:], brow_d_[:])
                grow = sp.tile([1, D], F32, tag="grow", name="grow")
                brow2 = sp.tile([1, D], F32, tag="brow2", name="brow2")
                nc.sync.dma_start(grow[:], g_d_[:])
                nc.sync.dma_start(brow2[:], b_d_[:])
                g_ = sp.tile([128, D], F32, tag="gb_g", name="gb_g")
                b_ = sp.tile([128, D], F32, tag="gb_b", name="gb_b")
                nc.gpsimd.partition_broadcast(g_[:], grow[:], channels=128)
                nc.gpsimd.partition_broadcast(b_[:], brow2[:], channels=128)
                if resid_is_dram:
                    rtiles = []
                    for t in range(TT):
                        rt = sp.tile([128, D], F32, tag=f"res{t}", name=f"res{t}")
                        nc.sync.dma_start(rt[:], resid[t * 128:(t + 1) * 128, :])
                        rtiles.append(rt)
                    resid = rtiles
                ps = [[pp.tile([128, CH], F32, tag=f"fc{t}_{f}", name=f"fc{t}_{f}")
                       for f in range(NOF)] for t in range(TT)]
                for k in range(nk):
                    wt = wp.tile([128, D], F32R, tag="w", name="w")
                    nc.sync.dma_start(wt[:], w_d_[k * 128:(k + 1) * 128, :])
                    for t in range(TT):
                        for f in range(NOF):
                            nc.tensor.matmul(
                                ps[t][f][:],
                                act_tiles[k][:, t * 128:(t + 1) * 128],
                                wt[:, f * CH:(f + 1) * CH],
                                start=(k == 0), stop=False)
                for t in range(TT):
                    for f in range(NOF):
                        nc.tensor.matmul(ps[t][f][:], ones128[:],
                                         brow_[:, f * CH:(f + 1) * CH],
                                         start=False, stop=True)
                for t in range(TT):
                    r = sp.tile([128, D], F32, tag="r", name="r")
                    rs = [sp.tile([128, 1], F32, tag=f"rs{f}", name=f"rs{f}")
                          for f in range(NOF)]
                    for f in range(NOF):
                        nc.vector.scalar_tensor_tensor(
                            r[:, f * CH:(f + 1) * CH], ps[t][f][:], 1.0,
                            resid[t][:, f * CH:(f + 1) * CH],
                            op0=ALU.mult, op1=ALU.add, accum_out=rs[f][:])
                    rowsum = sp.tile([128, 1], F32, tag="rowsum", name="rowsum")
                    if NOF == 2:
                        nc.vector.tensor_add(rowsum[:], rs[0][:], rs[1][:])
                    else:
                        nc.vector.tensor_copy(rowsum[:], rs[0][:])
                    negmean = sp.tile([128, 1], F32, tag="negmean", name="negmean")
                    nc.scalar.mul(negmean[:], rowsum[:], -1.0 / D)
                    xnl = sp.tile([128, D], F32, tag="xnl", name="xnl")
                    nc.scalar.activation(xnl[:], r[:], AF.Identity,
                                         bias=negmean[:])
                    xsq = sp.tile([128, D], F32, tag="xsq", name="xsq")
                    ssq = sp.tile([128, 1], F32, tag="ssq", name="ssq")
                    nc.scalar.activation(xsq[:], xnl[:], AF.Square,
                                         accum_out=ssq[:])
                    sd = sp.tile([128, 1], F32, tag="sd", name="sd")
                    nc.scalar.activation(sd[:], ssq[:], AF.Sqrt,
                                         bias=eps_t[:], scale=1.0 / D)
                    rstd = sp.tile([128, 1], F32, tag="rstd", name="rstd")
                    nc.vector.reciprocal(rstd[:], sd[:])
                    tmp = sp.tile([128, D], F32, tag="tmp", name="tmp")
                    nc.vector.scalar_tensor_tensor(
                        tmp[:], xnl[:], rstd[:], g_[:], op0=ALU.mult,
                        op1=ALU.mult)
                    nc.vector.tensor_add(out_tiles[t][:], tmp[:], b_[:])

        fc_ln(avfm, wsa_d, bsa_d, yblk_d, gb_d["g1"], gb_d["b1"], y1, NP,
              resid_is_dram=True)

        p_right = tc.alloc_tile_pool(name="p_right", bufs=1, side="right")
        yT12 = [p_right.tile([128, S], F32R, tag=f"yT12_{k}", name=f"yT12_{k}") for k in range(KD)]
        y2 = [p_right.tile([128, D], F32, tag=f"y2_{t}", name=f"y2_{t}") for t in range(TT)]

        def transpose_to(src_tiles, dst_tiles):
            with (
                tc.tile_pool(name="tp_ps", bufs=2, space="PSUM") as pp,
                tc.tile_pool(name="tp_sb", bufs=2) as sp,
            ):
                for t in range(TT):
                    srcr = sp.tile([128, D], F32R, tag="srcr", name="srcr")
                    nc.vector.tensor_copy(srcr[:], src_tiles[t][:])
                    for k in range(KD):
                        pst = pp.tile([128, 128], F32R, tag="tp", name="tp")
                        nc.tensor.transpose(pst[:],
                                            srcr[:, k * 128:(k + 1) * 128],
                                            ident[:])
                        nc.vector.tensor_copy(
                            dst_tiles[k][:, t * 128:(t + 1) * 128], pst[:])

        transpose_to(y1, yT12)

        # ------------------------- cross attention -------------------------
        with (
            tc.tile_pool(name="q2_w", bufs=2) as wp,
            tc.tile_pool(name="q2_ps", bufs=2, space="PSUM") as pp,
        ):
            for p in range(NP):
                psq = pp.tile([128, S], F32, tag="psq2", name="psq2")
                for k in range(KD):
                    wt = wp.tile([128, 128], F32R, tag="wq2", name="wq2")
                    nc.sync.dma_start(
                        wt[:], wq2_d[k * 128:(k + 1) * 128,
                                     p * 128:(p + 1) * 128])
                    nc.tensor.matmul(psq[:], wt[:], yT12[k][:, 0:S],
                                     start=(k == 0), stop=(k == KD - 1))
                nc.vector.tensor_scalar_add(qfm[p][:], psq[:],
                                            bq2T[:, p:p + 1])

        attention(qfm, k2fm_s, v2ext_s, avfm, causal=False)
        fc_ln(avfm, wca_d, bca_d, y1, gb_d["g2"], gb_d["b2"], y2, NP)
        transpose_to(y2, yT12)
        p_att.release()

        # ------------------------------ FFN ---------------------------------
        p_h = tc.alloc_tile_pool(name="p_h", bufs=1)
        hfm = [p_h.tile([128, S], F32R, tag=f"h{f}", name=f"h{f}") for f in range(FT)]
        with (
            tc.tile_pool(name="f1_w", bufs=4) as wp,
            tc.tile_pool(name="f1_ps", bufs=2, space="PSUM") as pp,
        ):
            for fg in range(FT // 4):
                psf = [pp.tile([128, S], F32, tag=f"psf{j}", name=f"psf{j}")
                       for j in range(4)]
                for k in range(KD):
                    wt = wp.tile([128, 512], F32R, tag="wff1", name="wff1")
                    nc.sync.dma_start(
                        wt[:], wff1_d[k * 128:(k + 1) * 128,
                                      fg * 512:(fg + 1) * 512])
                    for j in range(4):
                        nc.tensor.matmul(
                            psf[j][:], wt[:, j * 128:(j + 1) * 128],
                            yT12[k][:, 0:S], start=(k == 0),
                            stop=(k == KD - 1))
                for j in range(4):
                    f = fg * 4 + j
                    nc.scalar.activation(hfm[f][:], psf[j][:], AF.Relu,
                                         bias=bff1T[:, f:f + 1])
        out_f = [p_h.tile([128, D], F32, tag=f"out{t}", name=f"out{t}") for t in range(TT)]
        fc_ln(hfm, wff2_d, bff2_d, y2, gb_d["g3"], gb_d["b3"], out_f, FT)
        for t in range(TT):
            nc.sync.dma_start(out_d[t * 128:(t + 1) * 128, :], out_f[t][:])
        p_h.release()
        p_right.release()
        cpool.release()

    nc.compile()
    return nc


# ---------------------------------------------------------------- hosting ---
def make_inputs_for_core(full, b, o, D=1024, H=16, FF=4096, S=512, E=2048):
    HD = D // H
    KT = E // 128
    y = np.asarray(full["y"][b], dtype=np.float32)      # [E, D]
    x = np.asarray(full["x"][b], dtype=np.float32)
    perm = np.concatenate([np.arange(o, o + S), np.arange(0, o),
                           np.arange(o + S, E)])
    yT = np.ascontiguousarray(y.T[:, perm])
    xT = np.ascontiguousarray(x.T)
    kbias = np.zeros((128, KT), np.float32)
    idx = np.arange(E).reshape(KT, 128).T               # [128, KT]
    kbias[idx >= S + o] = -30.0

    qkv_w = np.asarray(full["qkv_w"], np.float32).reshape(D, H, 3 * HD)
    wq = np.ascontiguousarray(qkv_w[:, :, 0:HD].reshape(D, D))
    wk = np.ascontiguousarray(qkv_w[:, :, HD:2 * HD].reshape(D, D))
    wv = np.ascontiguousarray(qkv_w[:, :, 2 * HD:].reshape(D, D))
    qkv_b = np.asarray(full["qkv_b"], np.float32).reshape(H, 3 * HD)
    bq = qkv_b[:, 0:HD].reshape(D)
    bk = qkv_b[:, HD:2 * HD].reshape(D)
    bv = qkv_b[:, 2 * HD:].reshape(D)
    kv_w = np.asarray(full["kv_w"], np.float32).reshape(D, H, 2 * HD)
    wk2 = np.ascontiguousarray(kv_w[:, :, 0:HD].reshape(D, D))
    wv2 = np.ascontiguousarray(kv_w[:, :, HD:].reshape(D, D))
    kv_b = np.asarray(full["kv_b"], np.float32).reshape(H, 2 * HD)
    bk2 = kv_b[:, 0:HD].reshape(D)
    bv2 = kv_b[:, HD:].reshape(D)

    def colT(v):   # [D] -> [128, D//128] (partition-major per 128-tile)
        return np.ascontiguousarray(v.reshape(-1, 128).T.astype(np.float32))

    ozsb = np.zeros((128, H * 64), np.float32)
    for h in range(H):
        ozsb[:, h * 64:h * 64 + 32] = 1.0

    return {
        "yT": yT, "yblk": np.ascontiguousarray(y[o:o + S]), "xT": xT,
        "kbias": kbias, "ident": np.eye(128, dtype=np.float32),
        "ones128": np.ones((1, 128), np.float32), "ozsb": ozsb,
        "wq": wq, "wk": wk, "wv": wv,
        "wsa": np.asarray(full["sa_fc_w"], np.float32),
        "wq2": np.asarray(full["q_w"], np.float32), "wk2": wk2, "wv2": wv2,
        "wca": np.asarray(full["ca_fc_w"], np.float32),
        "wff1": np.asarray(full["ff1_w"], np.float32),
        "wff2": np.asarray(full["ff2_w"], np.float32),
        "bqT": colT(bq), "bkT": colT(bk),
        "bq2T": colT(np.asarray(full["q_b"], np.float32)), "bk2T": colT(bk2),
        "bff1T": colT(np.asarray(full["ff1_b"], np.float32)),
        "bv_r": bv.reshape(1, D), "bv2_r": bv2.reshape(1, D),
        "bsa_r": np.asarray(full["sa_fc_b"], np.float32).reshape(1, D),
        "bca_r": np.asarray(full["ca_fc_b"], np.float32).reshape(1, D),
        "bff2_r": np.asarray(full["ff2_b"], np.float32).reshape(1, D),
        "g1": np.asarray(full["g1"], np.float32).reshape(1, D),
        "b1": np.asarray(full["b1"], np.float32).reshape(1, D),
        "g2": np.asarray(full["g2"], np.float32).reshape(1, D),
        "b2": np.asarray(full["b2"], np.float32).reshape(1, D),
        "g3": np.asarray(full["g3"], np.float32).reshape(1, D),
        "b3": np.asarray(full["b3"], np.float32).reshape(1, D),
    }


# ------------------------------------------------------------------ runner --
def _install_neff_cache():
    from concourse import bass2jax
    if getattr(bass2jax, "_my_cache_installed", False):
        return
    os.makedirs(NEFF_CACHE, exist_ok=True)
    orig = bass2jax.compile_bir_kernel

    def cached(ant_bir_str, compile_dir_path, neff_name=None, **kw):
        key_bytes = ant_bir_str.encode() if isinstance(ant_bir_str, str) else ant_bir_str
        cpath = os.path.join(NEFF_CACHE,
                             hashlib.sha256(key_bytes).hexdigest() + ".neff")
        if os.path.exists(cpath):
            return cpath
        import shutil
        neff = orig(ant_bir_str, compile_dir_path, neff_name=neff_name, **kw)
        shutil.copy(neff, cpath)
        return cpath

    bass2jax.compile_bir_kernel = cached
    bass2jax._my_cache_installed = True


def run_spmd(nc, in_maps, n_cores, profile_dir=None):
    import jax
    from jax.sharding import Mesh, PartitionSpec
    from jax.experimental.shard_map import shard_map
    from concourse.bass2jax import (_bass_exec_p, partition_id_tensor,
                                    install_neuronx_cc_hook)
    _install_neff_cache()
    install_neuronx_cc_hook()

    partition_name = nc.partition_id_tensor.name if nc.partition_id_tensor else None
    in_names, out_names, out_avals, zero_outs = [], [], [], []
    for alloc in nc.m.functions[0].allocations:
        if not isinstance(alloc, mybir.MemoryLocationSet):
            continue
        name = alloc.memorylocations[0].name
        if alloc.kind == "ExternalInput":
            if name != partition_name:
                in_names.append(name)
        elif alloc.kind == "ExternalOutput":
            shape = tuple(alloc.tensor_shape)
            dtype = mybir.dt.np(alloc.dtype)
            out_names.append(name)
            out_avals.append(jax.core.ShapedArray(shape, dtype))
            zero_outs.append(np.zeros(shape, dtype))
    n_params = len(in_names)
    n_outs = len(out_avals)
    in_names.extend(out_names)
    if partition_name is not None:
        in_names.append(partition_name)
    donate = tuple(range(n_params, n_params + n_outs))

    def _body(*args):
        operands = list(args)
        if partition_name is not None:
            operands.append(partition_id_tensor())
        outs = _bass_exec_p.bind(
            *operands, out_avals=tuple(out_avals), in_names=tuple(in_names),
            out_names=tuple(out_names), lowering_input_output_aliases=(),
            sim_require_finite=True, sim_require_nnan=True, nc=nc)
        return tuple(outs)

    _body.__name__ = "u" + uuid.uuid4().hex[:12] + "_body"
    devices = jax.devices()[:n_cores]
    mesh = Mesh(np.asarray(devices), ("core",))
    sharded = jax.jit(
        shard_map(_body, mesh=mesh,
                  in_specs=(PartitionSpec("core"),) * (n_params + n_outs),
                  out_specs=(PartitionSpec("core"),) * n_outs,
                  check_rep=False),
        donate_argnums=donate, keep_unused=True)
    per_core = [[np.asarray(m[name]) for name in in_names[:n_params]]
                for m in in_maps]
    concat_in = [np.concatenate([per_core[c][i] for c in range(n_cores)], axis=0)
                 for i in range(n_params)]
    concat_zeros = [np.zeros((n_cores * z.shape[0], *z.shape[1:]), z.dtype)
                    for z in zero_outs]
    exec_ns = None
    if profile_dir is not None:
        from trn_agent_boot.trn_boot import _ntff_profile_via_ctypes
        if 'antenv.axon_hooks' not in sys.modules:
            mod = types.ModuleType('antenv.axon_hooks')
            _h = [None]
            mod.set_axon_ntff_profile_hook = lambda h: _h.__setitem__(0, h)
            mod.get_axon_ntff_profile_hook = lambda: _h[0]
            sys.modules['antenv.axon_hooks'] = mod
            import antenv
            antenv.axon_hooks = mod
        import antenv.axon_hooks as ah
        if ah.get_axon_ntff_profile_hook() is None:
            ah.set_axon_ntff_profile_hook(
                _ntff_profile_via_ctypes('/opt/axon/libaxon_pjrt.so'))
        hook = ah.get_axon_ntff_profile_hook()
        os.makedirs(profile_dir, exist_ok=True)
        compiled = sharded.lower(*concat_in, *concat_zeros).compile()
        with hook(profile_dir, [0]):
            out_arrs = compiled(*concat_in, *concat_zeros)
            out_arrs = [np.asarray(a) for a in out_arrs]
        exec_ns = _exec_time_from_ntff(profile_dir, nc)
    else:
        out_arrs = sharded(*concat_in, *concat_zeros)
        out_arrs = [np.asarray(a) for a in out_arrs]
    results = [
        {name: out_arrs[i].reshape(n_cores, *out_avals[i].shape)[c]
         for i, name in enumerate(out_names)}
        for c in range(n_cores)]
    return results, exec_ns


def _exec_time_from_ntff(profile_dir, nc):
    try:
        import gauge.profiler
        from concourse.bass_utils import _process_ntff_profile
        from concourse._compat import FishPath
        if not glob.glob(os.path.join(profile_dir, "*_body*.ntff")):
            return None
        profile = gauge.profiler.Profile(
            profile_path=FishPath(profile_dir), kernel_dev_mode=True,
            profile_on_exit=False, bass_kernel=nc.m, offline_processing=True,
            fname="*_body*", metadata={})
        r = _process_ntff_profile(profile, profile_dir, nc, [0], None, False,
                                  {}, False)
        return r.exec_time_ns
    except Exception:
        return None


_prog_cache = {}


def kernel(**inputs) -> np.ndarray:
    B, S_full, D = 2, 2048, 1024
    S, E = 512, 2048
    key = (D, S, E)
    if key not in _prog_cache:
        _prog_cache[key] = build_program(D=D, H=16, FF=4096, S=S, E=E,
                                         n_cores=8)
    nc = _prog_cache[key]
    in_maps = []
    for c in range(8):
        b, q = c // 4, c % 4
        in_maps.append(make_inputs_for_core(inputs, b, q * S))
    results, _ = run_spmd(nc, in_maps, 8)
    out = np.zeros((B, S_full, D), np.float32)
    for c in range(8):
        b, q = c // 4, c % 4
        out[b, q * S:(q + 1) * S] = results[c]["out_blk"]
    return out



# revision 25
# speedup vs baseline: 1.2893x; 1.2893x over previous
"""Decoder layer (self-attn + cross-attn + FFN, 3 post-LNs) on 8 TRN2 cores.

Sharding: core c -> (batch b = c//4, query block q = c%4 of 512 tokens).
Uniform SPMD program; causality via per-core data (permuted key order puts the
own block first, so the diagonal is always key-tiles [0, S/128) and
fully-masked keys get an exp bias of -30 from a per-core bias column).
All matmuls run in bf16 (fp32 PSUM accumulation). Self K/V and cross V live
in SBUF (no DRAM scratch roundtrip); V tiles carry a ones column per head so
the AV matmul also produces the softmax denominator at partition 64.
"""
import sys, os, uuid, hashlib, types, glob
sys.path.insert(0, '/opt/trn_rl_repo')
import numpy as np
import ml_dtypes

import concourse.bass as bass
import concourse.bacc as bacc
import concourse.mybir as mybir
import concourse.tile as tile

F32 = mybir.dt.float32
BF16 = mybir.dt.bfloat16
AF = mybir.ActivationFunctionType
ALU = mybir.AluOpType
BF16NP = ml_dtypes.bfloat16

NEFF_CACHE = os.environ.get("MY_NEFF_CACHE", "/root/.my_neff_cache")


# ---------------------------------------------------------------- program ---
def build_program(D=1024, H=16, FF=4096, S=512, E=2048, n_cores=8):
    HD = 64
    assert D == H * HD
    KD = D // 128          # contraction tiles over model dim
    NP = H // 2            # head pairs (=feature 128-tiles of q/k)
    TT = S // 128          # token tiles of own block
    KT = E // 128          # key tiles
    FT = FF // 128
    DIAG = S // 128        # first DIAG key tiles are the causal diagonal
    HW = 72                # head width in V tiles: 64 V + 1 ones + 7 pad
                           # (even/16B-aligned head stride for the PE
                           # stationary reads)
    CH = min(512, D)       # free-dim chunk over model features
    NOF = D // CH
    KC = min(512, E)       # key chunk
    TC = min(512, E)       # token chunk for projections
    NTC = E // TC
    HPC = CH // 64         # heads per V-projection chunk

    nc = bacc.Bacc("TRN2", target_bir_lowering=False, debug=False,
                   num_devices=n_cores)

    def din(name, shape, dt=BF16):
        return nc.dram_tensor(name, shape, dt, kind="ExternalInput")

    yT_d = din("yT", [D, E])
    yblk_d = din("yblk", [S, D], F32)
    xT_d = din("xT", [D, E])
    kbias_d = din("kbias", [128, KT], F32)
    ident_d = din("ident", [128, 128], mybir.dt.float32r)
    ones128_d = din("ones128", [1, 128])
    oneshw_d = din("oneshw", [128, H * (HW - 64)])
    wq_d, wk_d, wv_d = din("wq", [D, D]), din("wk", [D, D]), din("wv", [D, D])
    wsa_d = din("wsa", [D, D])
    wq2_d, wk2_d, wv2_d = din("wq2", [D, D]), din("wk2", [D, D]), din("wv2", [D, D])
    wca_d = din("wca", [D, D])
    wff1_d, wff2_d = din("wff1", [D, FF]), din("wff2", [FF, D])
    bqT_d, bkT_d = din("bqT", [128, NP], F32), din("bkT", [128, NP], F32)
    bq2T_d, bk2T_d = din("bq2T", [128, NP], F32), din("bk2T", [128, NP], F32)
    bff1T_d = din("bff1T", [128, FT], F32)
    bv_d, bv2_d = din("bv_r", [1, D]), din("bv2_r", [1, D])
    bsa_d, bca_d, bff2_d = din("bsa_r", [1, D]), din("bca_r", [1, D]), din("bff2_r", [1, D])
    gb_d = {n: din(n, [1, D], F32) for n in ("g1", "b1", "g2", "b2", "g3", "b3")}

    out_d = nc.dram_tensor("out_blk", [S, D], F32, kind="ExternalOutput")
    tap_kfm0_d = nc.dram_tensor("tap_kfm0", [128, E], BF16, kind="ExternalOutput")
    tap_vt0_d = nc.dram_tensor("tap_vt0", [128, H * 72], BF16, kind="ExternalOutput")
    tap_q0_d = nc.dram_tensor("tap_q0", [128, S], BF16, kind="ExternalOutput")
    tap_av0_d = nc.dram_tensor("tap_av0", [128, S], BF16, kind="ExternalOutput")
    tap_y10_d = nc.dram_tensor("tap_y10", [128, D], F32, kind="ExternalOutput")

    # cross-attention K streamed through DRAM (feature-major, big descriptors)
    k2fm_s = nc.dram_tensor("k2fm_s", [D, E], BF16)

    with tile.TileContext(nc) as tc:
        cpool = tc.alloc_tile_pool(name="const", bufs=1)
        ident = cpool.tile([128, 128], mybir.dt.float32r, tag="ident", name="ident")
        ones128 = cpool.tile([1, 128], BF16, tag="ones128", name="ones128")
        oneshw = cpool.tile([128, H * (HW - 64)], BF16, tag="oneshw", name="oneshw")
        kbias = cpool.tile([128, KT], F32, tag="kbias", name="kbias")
        eps_t = cpool.tile([128, 1], F32, tag="eps", name="eps")
        nc.vector.memset(eps_t[:], 1e-9)
        for t, d in [(ident, ident_d), (ones128, ones128_d),
                     (oneshw, oneshw_d), (kbias, kbias_d)]:
            nc.sync.dma_start(t[:], d[:])
        bqT = cpool.tile([128, NP], F32, tag="bqT", name="bqT")
        bkT = cpool.tile([128, NP], F32, tag="bkT", name="bkT")
        bq2T = cpool.tile([128, NP], F32, tag="bq2T", name="bq2T")
        bk2T = cpool.tile([128, NP], F32, tag="bk2T", name="bk2T")
        bff1T = cpool.tile([128, FT], F32, tag="bff1T", name="bff1T")
        for t, d in [(bqT, bqT_d), (bkT, bkT_d), (bq2T, bq2T_d),
                     (bk2T, bk2T_d), (bff1T, bff1T_d)]:
            nc.sync.dma_start(t[:], d[:])
        p_att = tc.alloc_tile_pool(name="p_att", bufs=1)
        qfm = [p_att.tile([128, S], BF16, tag=f"qfm{p}", name=f"qfm{p}") for p in range(NP)]
        avfm = [p_att.tile([128, S], BF16, tag=f"avfm{p}", name=f"avfm{p}") for p in range(NP)]
        y1 = [p_att.tile([128, D], F32, tag=f"y1_{t}", name=f"y1_{t}") for t in range(TT)]

        # self K/V SBUF pool (released after self-attention)
        p_kv1 = tc.alloc_tile_pool(name="p_kv1", bufs=1)
        kfm1 = [p_kv1.tile([128, E], BF16, tag=f"kfm1_{p}", name=f"kfm1_{p}")
                for p in range(NP)]
        vt1 = [p_kv1.tile([128, H * HW], BF16, tag=f"vt1_{t}", name=f"vt1_{t}")
               for t in range(KT)]
        # cross V SBUF pool (released after cross-attention)
        p_kv2 = tc.alloc_tile_pool(name="p_kv2", bufs=1, side="right")
        vt2 = [p_kv2.tile([128, H * HW], BF16, tag=f"vt2_{t}", name=f"vt2_{t}")
               for t in range(KT)]

        # ------------- chunked K/V/Q projections (src streamed) ------------
        def kvq_proj(*a, **kw):
            for _ in kvq_gen(*a, **kw):
                pass

        def kvq_gen(srcT_d, wq_, wk_, wv_, bq_, bk_, bv_, q_dst, kfm_sb,
                    kfm_dr, vt_dst, with_q, src_bufs=2, ps_bufs=2,
                    lean_v=False):
            with (
                tc.tile_pool(name="pj_src", bufs=src_bufs) as srp,
                tc.tile_pool(name="pj_wk", bufs=1) as wkp,
                tc.tile_pool(name="pj_wv", bufs=2) as wvp,
                tc.tile_pool(name="pj_ev", bufs=2) as evp,
                tc.tile_pool(name="pj_ps", bufs=ps_bufs, space="PSUM") as pp,
            ):
                # ones column per head in the V tiles (DVE copy from const)
                for tb in range(KT):
                    nc.vector.tensor_copy(
                        vt_dst[tb][:].rearrange("p (h c) -> p h c", c=HW)
                        [:, :, 64:HW],
                        oneshw[:].rearrange("p (h c) -> p h c", c=HW - 64))
                bvrow = evp.tile([1, D], BF16, tag="bvrow", name="bvrow")
                nc.sync.dma_start(bvrow[:], bv_[:])
                wk_t = []
                wv_t = []
                for k in range(KD):
                    t = wkp.tile([128, D], BF16, tag=f"wk{k}", name=f"wk{k}")
                    nc.sync.dma_start(t[:], wk_[k * 128:(k + 1) * 128, :])
                    wk_t.append(t)
                    t2 = wkp.tile([128, D], BF16, tag=f"wv{k}", name=f"wv{k}")
                    nc.sync.dma_start(t2[:], wv_[k * 128:(k + 1) * 128, :])
                    wv_t.append(t2)
                yield
                for tch in range(NTC):
                    src = []
                    for k in range(KD):
                        t = srp.tile([128, TC], BF16, tag=f"src{k}", name=f"src{k}")
                        nc.sync.dma_start(
                            t[:], srcT_d[k * 128:(k + 1) * 128,
                                         tch * TC:(tch + 1) * TC])
                        src.append(t)
                    # K projection, feature-major out
                    for p in range(NP):
                        ps = pp.tile([128, TC], F32, tag="psk", name="psk")
                        for k in range(KD):
                            nc.tensor.matmul(
                                ps[:], wk_t[k][:, p * 128:(p + 1) * 128],
                                src[k][:], start=(k == 0), stop=(k == KD - 1))
                        if kfm_sb is not None:
                            nc.vector.tensor_scalar_add(
                                kfm_sb[p][:, tch * TC:(tch + 1) * TC], ps[:],
                                bk_[:, p:p + 1])
                        else:
                            ev = evp.tile([128, TC], BF16, tag="kev", name="kev")
                            nc.vector.tensor_scalar_add(ev[:], ps[:],
                                                        bk_[:, p:p + 1])
                            nc.gpsimd.dma_start(
                                kfm_dr[p * 128:(p + 1) * 128,
                                       tch * TC:(tch + 1) * TC], ev[:])
                    # Q projection (own block = chunk 0, cols 0:S)
                    if with_q and tch == 0:
                        for p in range(NP):
                            ps = pp.tile([128, S], F32, tag="psq", name="psq")
                            for k in range(KD):
                                wqt = wvp.tile([128, 128], BF16, tag="wqs", name="wqs")
                                nc.sync.dma_start(
                                    wqt[:], wq_[k * 128:(k + 1) * 128,
                                                p * 128:(p + 1) * 128])
                                nc.tensor.matmul(
                                    ps[:], wqt[:],
                                    src[k][:, 0:S], start=(k == 0),
                                    stop=(k == KD - 1))
                            nc.vector.tensor_scalar_add(q_dst[p][:], ps[:],
                                                        bq_[:, p:p + 1])
                    # V projection, token-major out into SBUF V tiles
                    for tt_ in range(TC // 128):
                        kt_ = tch * (TC // 128) + tt_
                        if lean_v:
                            psv = []
                            for vf in range(NOF):
                                pv = pp.tile([128, CH], F32, tag="psv",
                                             name="psv")
                                for k in range(KD):
                                    nc.tensor.matmul(
                                        pv[:],
                                        src[k][:, tt_ * 128:(tt_ + 1) * 128],
                                        wv_t[k][:, vf * CH:(vf + 1) * CH],
                                        start=(k == 0), stop=False)
                                psv.append(pv)
                        else:
                            psv = [pp.tile([128, CH], F32, tag=f"psv{vf}",
                                           name=f"psv{vf}") for vf in range(NOF)]
                            for k in range(KD):
                                for vf in range(NOF):
                                    nc.tensor.matmul(
                                        psv[vf][:],
                                        src[k][:, tt_ * 128:(tt_ + 1) * 128],
                                        wv_t[k][:, vf * CH:(vf + 1) * CH],
                                        start=(k == 0), stop=False)
                        for vf in range(NOF):
                            nc.tensor.matmul(psv[vf][:], ones128[:],
                                             bvrow[:, vf * CH:(vf + 1) * CH],
                                             start=False, stop=True)
                            nc.vector.tensor_copy(
                                vt_dst[kt_][:]
                                .rearrange("p (h c) -> p h c", c=HW)
                                [:, vf * HPC:(vf + 1) * HPC, 0:64],
                                psv[vf][:].rearrange("p (h c) -> p h c", c=64))
                        yield

        kvq_proj(yT_d, wq_d, wk_d, wv_d, bqT, bkT, bv_d, qfm, kfm1, None,
                 vt1, with_q=True)
        nc.sync.dma_start(tap_kfm0_d[:], kfm1[0][:])
        nc.sync.dma_start(tap_vt0_d[:], vt1[0][:])
        nc.sync.dma_start(tap_q0_d[:], qfm[0][:])

        # ------------------------- attention -------------------------------
        def attention(*a, **kw):
            for _ in att_gen(*a, **kw):
                pass

        def att_gen(q_tiles, kfm_sb, kfm_dr, vt_src, out_tiles, causal,
                    ss_bufs=2, pav_bufs=2):
            with (
                tc.tile_pool(name="at_sb", bufs=3) as sp,
                tc.tile_pool(name="at_ps", bufs=ss_bufs, space="PSUM") as pp,
                tc.tile_pool(name="at_pav", bufs=pav_bufs, space="PSUM") as pav,
                tc.tile_pool(name="at_n", bufs=2) as npool,
            ):
                nkj = KC // 128
                for p in range(NP):
                    psA = pav.tile([128, S], F32, tag="psavA", name="psavA")
                    psB = pav.tile([128, S], F32, tag="psavB", name="psavB")
                    klt = None
                    pend = None   # skew-1: AV(kt-1) is emitted after scores(kt)

                    def emit_av(pr2_, kt_):
                        vl = vt_src[kt_]
                        nc.tensor.matmul(psA[0:65, :],
                                         vl[:, 2 * p * HW:2 * p * HW + 65],
                                         pr2_[:, 0:S], start=(kt_ == 0),
                                         stop=(kt_ == KT - 1))
                        nc.tensor.matmul(psB[0:65, :],
                                         vl[:, (2 * p + 1) * HW:(2 * p + 1) * HW + 65],
                                         pr2_[:, S:2 * S], start=(kt_ == 0),
                                         stop=(kt_ == KT - 1))

                    for kt in range(KT):
                        if kfm_sb is not None:
                            kl = kfm_sb[p]
                            ko = kt * 128
                        else:
                            kj = kt % nkj
                            if kj == 0:
                                ktc = kt // nkj
                                klt = sp.tile([128, KC], BF16, tag="kl", name="kl")
                                nc.sync.dma_start(
                                    klt[:], kfm_dr[p * 128:(p + 1) * 128,
                                                   ktc * KC:(ktc + 1) * KC])
                            kl = klt
                            ko = kj * 128
                        ss = pp.tile([128, 2 * S], F32, tag="ss", name="ss")
                        nc.tensor.matmul(
                            ss[:, 0:S], kl[0:64, ko:ko + 128],
                            q_tiles[p][0:64, :], start=True, stop=True,
                            tile_position=(0, 0))
                        nc.tensor.matmul(
                            ss[:, S:2 * S],
                            kl[64:128, ko:ko + 128],
                            q_tiles[p][64:128, :], start=True, stop=True,
                            tile_position=(64, 0))
                        if pend is not None:
                            emit_av(*pend)
                        pr2 = sp.tile([128, 2 * S], BF16, tag="pr2", name="pr2")
                        bias_ap = kbias[:, kt:kt + 1] if causal else 0.0
                        nc.scalar.activation(pr2[:], ss[:], AF.Exp,
                                             bias=bias_ap, scale=0.125)
                        if causal and kt < DIAG:
                            for half in range(2):
                                nc.gpsimd.affine_select(
                                    out=pr2[:, half * S:(half + 1) * S],
                                    in_=pr2[:, half * S:(half + 1) * S],
                                    compare_op=ALU.is_ge, fill=0.0,
                                    base=-kt * 128, channel_multiplier=-1,
                                    pattern=[[1, S]])
                        pend = (pr2, kt)
                    emit_av(*pend)
                    recA = npool.tile([1, S], F32, tag="recA", name="recA")
                    recB = npool.tile([1, S], F32, tag="recB", name="recB")
                    nc.vector.reciprocal(recA[:], psA[64:65, :])
                    nc.vector.reciprocal(recB[:], psB[64:65, :])
                    rbA = npool.tile([64, S], F32, tag="rbA", name="rbA")
                    rbB = npool.tile([64, S], F32, tag="rbB", name="rbB")
                    nc.gpsimd.partition_broadcast(rbA[:], recA[:], channels=64)
                    nc.gpsimd.partition_broadcast(rbB[:], recB[:], channels=64)
                    avB = npool.tile([64, S], BF16, tag="avB", name="avB")
                    nc.vector.tensor_mul(out_tiles[p][0:64, :], psA[0:64, :],
                                         rbA[:])
                    nc.vector.tensor_mul(avB[:], psB[0:64, :], rbB[:])
                    nc.sync.dma_start(out_tiles[p][64:128, :], avB[:])
                    yield
                yield  # hold pools open until the driver drains us last

        # Interleave self-attention (ACT-heavy) with cross K/V projection
        # (PE-heavy, independent) so the PE fills attention's exp-wait gaps.
        g_att = att_gen(qfm, kfm1, None, vt1, avfm, causal=True,
                        ss_bufs=2, pav_bufs=1)
        g_cross = kvq_gen(xT_d, None, wk2_d, wv2_d, None, bk2T, bv2_d, None,
                          None, k2fm_s, vt2, with_q=False, src_bufs=1,
                          ps_bufs=1, lean_v=True)
        adv = 0
        next(g_att)             # opens att pools first (released last)
        while True:
            try:
                next(g_cross)
            except StopIteration:
                break
            if adv < NP:        # never exhaust g_att while g_cross is open
                next(g_att)
                adv += 1
        for _ in g_att:
            pass
        nc.sync.dma_start(tap_av0_d[:], avfm[0][:])
        p_kv1.release()

        # ------------------ fc + residual + LN (token-major) ---------------
        def fc_ln(act_tiles, w_d_, brow_d_, resid, g_d_, b_d_, out_tiles, nk,
                  resid_is_dram=False):
            with (
                tc.tile_pool(name="fc_w", bufs=3) as wp,
                tc.tile_pool(name="fc_ps", bufs=1, space="PSUM") as pp,
                tc.tile_pool(name="fc_sb", bufs=1) as sp,
            ):
                brow_ = sp.tile([1, D], BF16, tag="brow", name="brow")
                nc.sync.dma_start(brow_[:], brow_d_[:])
                grow = sp.tile([1, D], F32, tag="grow", name="grow")
                brow2 = sp.tile([1, D], F32, tag="brow2", name="brow2")
                nc.sync.dma_start(grow[:], g_d_[:])
                nc.sync.dma_start(brow2[:], b_d_[:])
                g_ = sp.tile([128, D], F32, tag="gb_g", name="gb_g")
                b_ = sp.tile([128, D], F32, tag="gb_b", name="gb_b")
                nc.gpsimd.partition_broadcast(g_[:], grow[:], channels=128)
                nc.gpsimd.partition_broadcast(b_[:], brow2[:], channels=128)
                if resid_is_dram:
                    rtiles = []
                    for t in range(TT):
                        rt = sp.tile([128, D], F32, tag=f"res{t}", name=f"res{t}")
                        nc.sync.dma_start(rt[:], resid[t * 128:(t + 1) * 128, :])
                        rtiles.append(rt)
                    resid = rtiles
                ps = [[pp.tile([128, CH], F32, tag=f"fc{t}_{f}", name=f"fc{t}_{f}")
                       for f in range(NOF)] for t in range(TT)]
                for k in range(nk):
                    wt = wp.tile([128, D], BF16, tag="w", name="w")
                    nc.sync.dma_start(wt[:], w_d_[k * 128:(k + 1) * 128, :])
                    for t in range(TT):
                        for f in range(NOF):
                            nc.tensor.matmul(
                                ps[t][f][:],
                                act_tiles[k][:, t * 128:(t + 1) * 128],
                                wt[:, f * CH:(f + 1) * CH],
                                start=(k == 0), stop=False)
                for t in range(TT):
                    for f in range(NOF):
                        nc.tensor.matmul(ps[t][f][:], ones128[:],
                                         brow_[:, f * CH:(f + 1) * CH],
                                         start=False, stop=True)
                for t in range(TT):
                    r = sp.tile([128, D], F32, tag="r", name="r")
                    rs = [sp.tile([128, 1], F32, tag=f"rs{f}", name=f"rs{f}")
                          for f in range(NOF)]
                    for f in range(NOF):
                        nc.vector.scalar_tensor_tensor(
                            r[:, f * CH:(f + 1) * CH], ps[t][f][:], 1.0,
                            resid[t][:, f * CH:(f + 1) * CH],
                            op0=ALU.mult, op1=ALU.add, accum_out=rs[f][:])
                    rowsum = sp.tile([128, 1], F32, tag="rowsum", name="rowsum")
                    if NOF == 2:
                        nc.vector.tensor_add(rowsum[:], rs[0][:], rs[1][:])
                    else:
                        nc.vector.tensor_copy(rowsum[:], rs[0][:])
                    negmean = sp.tile([128, 1], F32, tag="negmean", name="negmean")
                    nc.scalar.mul(negmean[:], rowsum[:], -1.0 / D)
                    xnl = sp.tile([128, D], F32, tag="xnl", name="xnl")
                    nc.scalar.activation(xnl[:], r[:], AF.Identity,
                                         bias=negmean[:])
                    xsq = sp.tile([128, D], F32, tag="xsq", name="xsq")
                    ssq = sp.tile([128, 1], F32, tag="ssq", name="ssq")
                    nc.scalar.activation(xsq[:], xnl[:], AF.Square,
                                         accum_out=ssq[:])
                    sd = sp.tile([128, 1], F32, tag="sd", name="sd")
                    nc.scalar.activation(sd[:], ssq[:], AF.Sqrt,
                                         bias=eps_t[:], scale=1.0 / D)
                    rstd = sp.tile([128, 1], F32, tag="rstd", name="rstd")
                    nc.vector.reciprocal(rstd[:], sd[:])
                    tmp = sp.tile([128, D], F32, tag="tmp", name="tmp")
                    nc.vector.scalar_tensor_tensor(
                        tmp[:], xnl[:], rstd[:], g_[:], op0=ALU.mult,
                        op1=ALU.mult)
                    nc.vector.tensor_add(out_tiles[t][:], tmp[:], b_[:])

        fc_ln(avfm, wsa_d, bsa_d, yblk_d, gb_d["g1"], gb_d["b1"], y1, NP,
              resid_is_dram=True)
        nc.sync.dma_start(tap_y10_d[:], y1[0][:])

        p_right = tc.alloc_tile_pool(name="p_right", bufs=1, side="right")
        yT12 = [p_right.tile([128, S], BF16, tag=f"yT12_{k}", name=f"yT12_{k}") for k in range(KD)]
        y2 = [p_right.tile([128, D], F32, tag=f"y2_{t}", name=f"y2_{t}") for t in range(TT)]

        def transpose_to(src_tiles, dst_tiles):
            with (
                tc.tile_pool(name="tp_ps", bufs=2, space="PSUM") as pp,
                tc.tile_pool(name="tp_sb", bufs=2) as sp,
            ):
                for t in range(TT):
                    srcr = sp.tile([128, D], mybir.dt.float32r, tag="srcr", name="srcr")
                    nc.vector.tensor_copy(srcr[:], src_tiles[t][:])
                    for k in range(KD):
                        pst = pp.tile([128, 128], mybir.dt.float32r, tag="tp", name="tp")
                        nc.tensor.transpose(pst[:],
                                            srcr[:, k * 128:(k + 1) * 128],
                                            ident[:])
                        nc.vector.tensor_copy(
                            dst_tiles[k][:, t * 128:(t + 1) * 128], pst[:])

        transpose_to(y1, yT12)

        # ------------------------- cross attention -------------------------
        with (
            tc.tile_pool(name="q2_w", bufs=2) as wp,
            tc.tile_pool(name="q2_ps", bufs=2, space="PSUM") as pp,
        ):
            for p in range(NP):
                psq = pp.tile([128, S], F32, tag="psq2", name="psq2")
                for k in range(KD):
                    wt = wp.tile([128, 128], BF16, tag="wq2", name="wq2")
                    nc.sync.dma_start(
                        wt[:], wq2_d[k * 128:(k + 1) * 128,
                                     p * 128:(p + 1) * 128])
                    nc.tensor.matmul(psq[:], wt[:], yT12[k][:, 0:S],
                                     start=(k == 0), stop=(k == KD - 1))
                nc.vector.tensor_scalar_add(qfm[p][:], psq[:],
                                            bq2T[:, p:p + 1])

        attention(qfm, None, k2fm_s, vt2, avfm, causal=False)
        fc_ln(avfm, wca_d, bca_d, y1, gb_d["g2"], gb_d["b2"], y2, NP)
        transpose_to(y2, yT12)
        p_att.release()

        # ------------------------------ FFN ---------------------------------
        p_h = tc.alloc_tile_pool(name="p_h", bufs=1)
        hfm = [p_h.tile([128, S], BF16, tag=f"h{f}", name=f"h{f}") for f in range(FT)]
        with (
            tc.tile_pool(name="f1_w", bufs=4) as wp,
            tc.tile_pool(name="f1_ps", bufs=2, space="PSUM") as pp,
        ):
            for fg in range(FT // 4):
                psf = [pp.tile([128, S], F32, tag=f"psf{j}", name=f"psf{j}")
                       for j in range(4)]
                for k in range(KD):
                    wt = wp.tile([128, 512], BF16, tag="wff1", name="wff1")
                    nc.sync.dma_start(
                        wt[:], wff1_d[k * 128:(k + 1) * 128,
                                      fg * 512:(fg + 1) * 512])
                    for j in range(4):
                        nc.tensor.matmul(
                            psf[j][:], wt[:, j * 128:(j + 1) * 128],
                            yT12[k][:, 0:S], start=(k == 0),
                            stop=(k == KD - 1))
                for j in range(4):
                    f = fg * 4 + j
                    nc.scalar.activation(hfm[f][:], psf[j][:], AF.Relu,
                                         bias=bff1T[:, f:f + 1])
        out_f = [p_h.tile([128, D], F32, tag=f"out{t}", name=f"out{t}") for t in range(TT)]
        fc_ln(hfm, wff2_d, bff2_d, y2, gb_d["g3"], gb_d["b3"], out_f, FT)
        for t in range(TT):
            nc.sync.dma_start(out_d[t * 128:(t + 1) * 128, :], out_f[t][:])
        p_h.release()
        p_right.release()
        p_kv2.release()
        cpool.release()

    nc.compile()
    return nc


# ---------------------------------------------------------------- hosting ---
def make_inputs_for_core(full, b, o, D=1024, H=16, FF=4096, S=512, E=2048):
    HD = D // H
    KT = E // 128
    y = np.asarray(full["y"][b], dtype=np.float32)      # [E, D]
    x = np.asarray(full["x"][b], dtype=np.float32)
    perm = np.concatenate([np.arange(o, o + S), np.arange(0, o),
                           np.arange(o + S, E)])
    yT = np.ascontiguousarray(y.T[:, perm])
    xT = np.ascontiguousarray(x.T)
    kbias = np.zeros((128, KT), np.float32)
    idx = np.arange(E).reshape(KT, 128).T               # [128, KT]
    kbias[idx >= S + o] = -30.0

    qkv_w = np.asarray(full["qkv_w"], np.float32).reshape(D, H, 3 * HD)
    wq = np.ascontiguousarray(qkv_w[:, :, 0:HD].reshape(D, D))
    wk = np.ascontiguousarray(qkv_w[:, :, HD:2 * HD].reshape(D, D))
    wv = np.ascontiguousarray(qkv_w[:, :, 2 * HD:].reshape(D, D))
    qkv_b = np.asarray(full["qkv_b"], np.float32).reshape(H, 3 * HD)
    bq = qkv_b[:, 0:HD].reshape(D)
    bk = qkv_b[:, HD:2 * HD].reshape(D)
    bv = qkv_b[:, 2 * HD:].reshape(D)
    kv_w = np.asarray(full["kv_w"], np.float32).reshape(D, H, 2 * HD)
    wk2 = np.ascontiguousarray(kv_w[:, :, 0:HD].reshape(D, D))
    wv2 = np.ascontiguousarray(kv_w[:, :, HD:].reshape(D, D))
    kv_b = np.asarray(full["kv_b"], np.float32).reshape(H, 2 * HD)
    bk2 = kv_b[:, 0:HD].reshape(D)
    bv2 = kv_b[:, HD:].reshape(D)

    def colT(v):   # [D] -> [128, D//128] (partition-major per 128-tile)
        return np.ascontiguousarray(v.reshape(-1, 128).T.astype(np.float32))

    def bf(a):
        return np.ascontiguousarray(np.asarray(a, np.float32)).astype(BF16NP)

    return {
        "yT": bf(yT), "yblk": np.ascontiguousarray(y[o:o + S]), "xT": bf(xT),
        "kbias": kbias, "ident": np.eye(128, dtype=np.float32),
        "ones128": bf(np.ones((1, 128), np.float32)),
        "oneshw": bf(np.concatenate([np.ones((128, H, 1), np.float32), np.zeros((128, H, 7), np.float32)], -1).reshape(128, H * 8)),
        "wq": bf(wq), "wk": bf(wk), "wv": bf(wv),
        "wsa": bf(full["sa_fc_w"]),
        "wq2": bf(full["q_w"]), "wk2": bf(wk2), "wv2": bf(wv2),
        "wca": bf(full["ca_fc_w"]),
        "wff1": bf(full["ff1_w"]),
        "wff2": bf(full["ff2_w"]),
        "bqT": colT(bq), "bkT": colT(bk),
        "bq2T": colT(np.asarray(full["q_b"], np.float32)), "bk2T": colT(bk2),
        "bff1T": colT(np.asarray(full["ff1_b"], np.float32)),
        "bv_r": bf(bv.reshape(1, D)), "bv2_r": bf(bv2.reshape(1, D)),
        "bsa_r": bf(np.asarray(full["sa_fc_b"], np.float32).reshape(1, D)),
        "bca_r": bf(np.asarray(full["ca_fc_b"], np.float32).reshape(1, D)),
        "bff2_r": bf(np.asarray(full["ff2_b"], np.float32).reshape(1, D)),
        "g1": np.asarray(full["g1"], np.float32).reshape(1, D),
        "b1": np.asarray(full["b1"], np.float32).reshape(1, D),
        "g2": np.asarray(full["g2"], np.float32).reshape(1, D),
        "b2": np.asarray(full["b2"], np.float32).reshape(1, D),
        "g3": np.asarray(full["g3"], np.float32).reshape(1, D),
        "b3": np.asarray(full["b3"], np.float32).reshape(1, D),
    }


# ------------------------------------------------------------------ runner --
def _install_neff_cache():
    from concourse import bass2jax
    if getattr(bass2jax, "_my_cache_installed", False):
        return
    os.makedirs(NEFF_CACHE, exist_ok=True)
    orig = bass2jax.compile_bir_kernel

    def cached(ant_bir_str, compile_dir_path, neff_name=None, **kw):
        key_bytes = ant_bir_str.encode() if isinstance(ant_bir_str, str) else ant_bir_str
        cpath = os.path.join(NEFF_CACHE,
                             hashlib.sha256(key_bytes).hexdigest() + ".neff")
        if os.path.exists(cpath):
            return cpath
        import shutil
        neff = orig(ant_bir_str, compile_dir_path, neff_name=neff_name, **kw)
        shutil.copy(neff, cpath)
        return cpath

    bass2jax.compile_bir_kernel = cached
    bass2jax._my_cache_installed = True


def run_spmd(nc, in_maps, n_cores, profile_dir=None):
    import jax
    from jax.sharding import Mesh, PartitionSpec
    from jax.experimental.shard_map import shard_map
    from concourse.bass2jax import (_bass_exec_p, partition_id_tensor,
                                    install_neuronx_cc_hook)
    _install_neff_cache()
    install_neuronx_cc_hook()

    partition_name = nc.partition_id_tensor.name if nc.partition_id_tensor else None
    in_names, out_names, out_avals, zero_outs = [], [], [], []
    for alloc in nc.m.functions[0].allocations:
        if not isinstance(alloc, mybir.MemoryLocationSet):
            continue
        name = alloc.memorylocations[0].name
        if alloc.kind == "ExternalInput":
            if name != partition_name:
                in_names.append(name)
        elif alloc.kind == "ExternalOutput":
            shape = tuple(alloc.tensor_shape)
            dtype = mybir.dt.np(alloc.dtype)
            out_names.append(name)
            out_avals.append(jax.core.ShapedArray(shape, dtype))
            zero_outs.append(np.zeros(shape, dtype))
    n_params = len(in_names)
    n_outs = len(out_avals)
    in_names.extend(out_names)
    if partition_name is not None:
        in_names.append(partition_name)
    donate = tuple(range(n_params, n_params + n_outs))

    def _body(*args):
        operands = list(args)
        if partition_name is not None:
            operands.append(partition_id_tensor())
        outs = _bass_exec_p.bind(
            *operands, out_avals=tuple(out_avals), in_names=tuple(in_names),
            out_names=tuple(out_names), lowering_input_output_aliases=(),
            sim_require_finite=True, sim_require_nnan=True, nc=nc)
        return tuple(outs)

    _body.__name__ = "u" + uuid.uuid4().hex[:12] + "_body"
    devices = jax.devices()[:n_cores]
    mesh = Mesh(np.asarray(devices), ("core",))
    sharded = jax.jit(
        shard_map(_body, mesh=mesh,
                  in_specs=(PartitionSpec("core"),) * (n_params + n_outs),
                  out_specs=(PartitionSpec("core"),) * n_outs,
                  check_rep=False),
        donate_argnums=donate, keep_unused=True)
    per_core = [[np.asarray(m[name]) for name in in_names[:n_params]]
                for m in in_maps]
    concat_in = [np.concatenate([per_core[c][i] for c in range(n_cores)], axis=0)
                 for i in range(n_params)]
    concat_zeros = [np.zeros((n_cores * z.shape[0], *z.shape[1:]), z.dtype)
                    for z in zero_outs]
    exec_ns = None
    if profile_dir is not None:
        from trn_agent_boot.trn_boot import _ntff_profile_via_ctypes
        if 'antenv.axon_hooks' not in sys.modules:
            mod = types.ModuleType('antenv.axon_hooks')
            _h = [None]
            mod.set_axon_ntff_profile_hook = lambda h: _h.__setitem__(0, h)
            mod.get_axon_ntff_profile_hook = lambda: _h[0]
            sys.modules['antenv.axon_hooks'] = mod
            import antenv
            antenv.axon_hooks = mod
        import antenv.axon_hooks as ah
        if ah.get_axon_ntff_profile_hook() is None:
            ah.set_axon_ntff_profile_hook(
                _ntff_profile_via_ctypes('/opt/axon/libaxon_pjrt.so'))
        hook = ah.get_axon_ntff_profile_hook()
        os.makedirs(profile_dir, exist_ok=True)
        compiled = sharded.lower(*concat_in, *concat_zeros).compile()
        with hook(profile_dir, [0]):
            out_arrs = compiled(*concat_in, *concat_zeros)
            out_arrs = [np.asarray(a) for a in out_arrs]
        exec_ns = _exec_time_from_ntff(profile_dir, nc)
    else:
        out_arrs = sharded(*concat_in, *concat_zeros)
        out_arrs = [np.asarray(a) for a in out_arrs]
    results = [
        {name: out_arrs[i].reshape(n_cores, *out_avals[i].shape)[c]
         for i, name in enumerate(out_names)}
        for c in range(n_cores)]
    return results, exec_ns


def _exec_time_from_ntff(profile_dir, nc):
    try:
        import gauge.profiler
        from concourse.bass_utils import _process_ntff_profile
        from concourse._compat import FishPath
        if not glob.glob(os.path.join(profile_dir, "*_body*.ntff")):
            return None
        profile = gauge.profiler.Profile(
            profile_path=FishPath(profile_dir), kernel_dev_mode=True,
            profile_on_exit=False, bass_kernel=nc.m, offline_processing=True,
            fname="*_body*", metadata={})
        r = _process_ntff_profile(profile, profile_dir, nc, [0], None, False,
                                  {}, False)
        return r.exec_time_ns
    except Exception:
        return None


_prog_cache = {}


def kernel(**inputs) -> np.ndarray:
    B, S_full, D = 2, 2048, 1024
    S, E = 512, 2048
    key = (D, S, E)
    if key not in _prog_cache:
        _prog_cache[key] = build_program(D=D, H=16, FF=4096, S=S, E=E,
                                         n_cores=8)
    nc = _prog_cache[key]
    in_maps = []
    for c in range(8):
        b, q = c // 4, c % 4
        in_maps.append(make_inputs_for_core(inputs, b, q * S))
    results, _ = run_spmd(nc, in_maps, 8)
    out = np.zeros((B, S_full, D), np.float32)
    for c in range(8):
        b, q = c // 4, c % 4
        out[b, q * S:(q + 1) * S] = results[c]["out_blk"]
    return out


# revision 37
# speedup vs baseline: 1.5466x; 1.1995x over previous
"""Decoder layer (self-attn + cross-attn + FFN, 3 post-LNs) on 8 TRN2 cores.

Sharding: core c -> (batch b = c//4, query block q = c%4 of 512 tokens).
Uniform SPMD program; causality via per-core data (permuted key order puts the
own block first, so the diagonal is always key-tiles [0, S/128) and
fully-masked keys get an exp bias of -30 from a per-core bias column).
All matmuls run in bf16 (fp32 PSUM accumulation). Self K/V and cross V live
in SBUF (no DRAM scratch roundtrip); V tiles carry a ones column per head so
the AV matmul also produces the softmax denominator at partition 64.
"""
import sys, os, uuid, hashlib, types, glob
sys.path.insert(0, '/opt/trn_rl_repo')
import numpy as np
import ml_dtypes

import concourse.bass as bass
import concourse.bacc as bacc
import concourse.mybir as mybir
import concourse.tile as tile

F32 = mybir.dt.float32
BF16 = mybir.dt.bfloat16
AF = mybir.ActivationFunctionType
ALU = mybir.AluOpType
BF16NP = ml_dtypes.bfloat16

NEFF_CACHE = os.environ.get("MY_NEFF_CACHE", "/root/.my_neff_cache")


# ---------------------------------------------------------------- program ---
def build_program(D=1024, H=16, FF=4096, S=512, E=2048, n_cores=8):
    HD = 64
    assert D == H * HD
    KD = D // 128          # contraction tiles over model dim
    NP = H // 2            # head pairs (=feature 128-tiles of q/k)
    TT = S // 128          # token tiles of own block
    KT = E // 128          # key tiles
    FT = FF // 128
    DIAG = S // 128        # first DIAG key tiles are the causal diagonal
    HW = 72                # head width in V tiles: 64 V + 1 ones + 7 pad
                           # (even/16B-aligned head stride for the PE
                           # stationary reads)
    CH = min(512, D)       # free-dim chunk over model features
    NOF = D // CH
    KC = min(512, E)       # key chunk
    TC = min(512, E)       # token chunk for projections
    NTC = E // TC
    HPC = CH // 64         # heads per V-projection chunk
    NB = E // S            # cores per batch (query blocks)

    nc = bacc.Bacc("TRN2", target_bir_lowering=False, debug=False,
                   num_devices=n_cores)

    def din(name, shape, dt=BF16):
        return nc.dram_tensor(name, shape, dt, kind="ExternalInput")

    yT_d = din("yT", [D, E])
    yTq_d = din("yTq", [D, S])
    yblk_d = din("yblk", [S, D], F32)
    xT_d = din("xT", [D, E])
    kbias_d = din("kbias", [128, KT], F32)
    ident_d = din("ident", [128, 128], mybir.dt.float32r)
    ones128_d = din("ones128", [1, 128])
    oneshw_d = din("oneshw", [128, H * (HW - 64)])
    wq_d, wk_d, wv_d = din("wq", [D, D]), din("wk", [D, D]), din("wv", [D, D])
    wsa_d = din("wsa", [D, D])
    wq2_d, wk2_d, wv2_d = din("wq2", [D, D]), din("wk2", [D, D]), din("wv2", [D, D])
    wca_d = din("wca", [D, D])
    wff1_d, wff2_d = din("wff1", [D, FF]), din("wff2", [FF, D])
    bqT_d, bkT_d = din("bqT", [128, NP], F32), din("bkT", [128, NP], F32)
    bq2T_d, bk2T_d = din("bq2T", [128, NP], F32), din("bk2T", [128, NP], F32)
    bff1T_d = din("bff1T", [128, FT], F32)
    bv_d, bv2_d = din("bv_r", [1, D]), din("bv2_r", [1, D])
    bsa_d, bca_d, bff2_d = din("bsa_r", [1, D]), din("bca_r", [1, D]), din("bff2_r", [1, D])
    gb_d = {n: din(n, [1, D], F32) for n in ("g1", "b1", "g2", "b2", "g3", "b3")}

    out_d = nc.dram_tensor("out_blk", [S, D], F32, kind="ExternalOutput")

    # cross-attention K streamed through DRAM (feature-major, big descriptors)
    k2fm_s = nc.dram_tensor("k2fm_s", [D, E], BF16)

    with tile.TileContext(nc) as tc:
        cpool = tc.alloc_tile_pool(name="const", bufs=1)
        ident = cpool.tile([128, 128], mybir.dt.float32r, tag="ident", name="ident")
        ones128 = cpool.tile([1, 128], BF16, tag="ones128", name="ones128")
        oneshw = cpool.tile([128, H * (HW - 64)], BF16, tag="oneshw", name="oneshw")
        kbias = cpool.tile([128, KT], F32, tag="kbias", name="kbias")
        eps_t = cpool.tile([128, 1], F32, tag="eps", name="eps")
        nc.vector.memset(eps_t[:], 1e-9)
        for t, d in [(ident, ident_d), (ones128, ones128_d),
                     (oneshw, oneshw_d), (kbias, kbias_d)]:
            nc.sync.dma_start(t[:], d[:])
        bqT = cpool.tile([128, NP], F32, tag="bqT", name="bqT")
        bkT = cpool.tile([128, NP], F32, tag="bkT", name="bkT")
        bq2T = cpool.tile([128, NP], F32, tag="bq2T", name="bq2T")
        bk2T = cpool.tile([128, NP], F32, tag="bk2T", name="bk2T")
        bff1T = cpool.tile([128, FT], F32, tag="bff1T", name="bff1T")
        for t, d in [(bqT, bqT_d), (bkT, bkT_d), (bq2T, bq2T_d),
                     (bk2T, bk2T_d), (bff1T, bff1T_d)]:
            nc.sync.dma_start(t[:], d[:])
        p_att = tc.alloc_tile_pool(name="p_att", bufs=1)
        qfm = [p_att.tile([128, S], BF16, tag=f"qfm{p}", name=f"qfm{p}") for p in range(NP)]
        avfm = [p_att.tile([128, S], BF16, tag=f"avfm{p}", name=f"avfm{p}") for p in range(NP)]
        y1 = [p_att.tile([128, D], F32, tag=f"y1_{t}", name=f"y1_{t}") for t in range(TT)]

        # self K/V SBUF pool (released after self-attention)
        p_kv1 = tc.alloc_tile_pool(name="p_kv1", bufs=1)
        kfm1 = [p_kv1.tile([128, E], BF16, tag=f"kfm1_{p}", name=f"kfm1_{p}")
                for p in range(NP)]
        vt1 = [p_kv1.tile([128, H * HW], BF16, tag=f"vt1_{t}", name=f"vt1_{t}")
               for t in range(KT)]
        # cross V SBUF pool (released after cross-attention)
        p_kv2 = tc.alloc_tile_pool(name="p_kv2", bufs=1, side="right")
        vt2 = [p_kv2.tile([128, H * HW], BF16, tag=f"vt2_{t}", name=f"vt2_{t}")
               for t in range(KT)]

        # ------------- chunked K/V/Q projections (src streamed) ------------
        def kvq_proj(*a, **kw):
            for _ in kvq_gen(*a, **kw):
                pass

        def kvq_gen(srcT_d, wq_, wk_, wv_, bq_, bk_, bv_, q_dst, kfm_sb,
                    kfm_dr, vt_dst, with_q, src_bufs=2, ps_bufs=2,
                    lean_v=False):
            with (
                tc.tile_pool(name="pj_src", bufs=src_bufs) as srp,
                tc.tile_pool(name="pj_wk", bufs=1) as wkp,
                tc.tile_pool(name="pj_wv", bufs=2) as wvp,
                tc.tile_pool(name="pj_ev", bufs=2) as evp,
                tc.tile_pool(name="pj_ps", bufs=ps_bufs, space="PSUM") as pp,
            ):
                # ones column per head in the V tiles (DVE copy from const)
                for tb in range(KT):
                    nc.vector.tensor_copy(
                        vt_dst[tb][:].rearrange("p (h c) -> p h c", c=HW)
                        [:, :, 64:HW],
                        oneshw[:].rearrange("p (h c) -> p h c", c=HW - 64))
                bvrow = evp.tile([1, D], BF16, tag="bvrow", name="bvrow")
                nc.sync.dma_start(bvrow[:], bv_[:])
                wk_t = []
                wv_t = []
                for k in range(KD):
                    t = wkp.tile([128, D], BF16, tag=f"wk{k}", name=f"wk{k}")
                    nc.sync.dma_start(t[:], wk_[k * 128:(k + 1) * 128, :])
                    wk_t.append(t)
                    t2 = wkp.tile([128, D], BF16, tag=f"wv{k}", name=f"wv{k}")
                    nc.sync.dma_start(t2[:], wv_[k * 128:(k + 1) * 128, :])
                    wv_t.append(t2)
                yield
                for tch in range(NTC):
                    src = []
                    for k in range(KD):
                        t = srp.tile([128, TC], BF16, tag=f"src{k}", name=f"src{k}")
                        nc.sync.dma_start(
                            t[:], srcT_d[k * 128:(k + 1) * 128,
                                         tch * TC:(tch + 1) * TC])
                        src.append(t)
                    # K projection, feature-major out
                    for p in range(NP):
                        ps = pp.tile([128, TC], F32, tag="psk", name="psk")
                        for k in range(KD):
                            nc.tensor.matmul(
                                ps[:], wk_t[k][:, p * 128:(p + 1) * 128],
                                src[k][:], start=(k == 0), stop=(k == KD - 1))
                        if kfm_sb is not None:
                            nc.vector.tensor_scalar_add(
                                kfm_sb[p][:, tch * TC:(tch + 1) * TC], ps[:],
                                bk_[:, p:p + 1])
                        else:
                            ev = evp.tile([128, TC], BF16, tag="kev", name="kev")
                            nc.vector.tensor_scalar_add(ev[:], ps[:],
                                                        bk_[:, p:p + 1])
                            nc.gpsimd.dma_start(
                                kfm_dr[p * 128:(p + 1) * 128,
                                       tch * TC:(tch + 1) * TC], ev[:])
                    # Q projection (own tiles, from the slot-ordered source)
                    if with_q and tch == 0:
                        qsrc = []
                        for k in range(KD):
                            t = srp.tile([128, S], BF16, tag=f"qsrc{k}",
                                         name=f"qsrc{k}")
                            nc.sync.dma_start(
                                t[:], yTq_d[k * 128:(k + 1) * 128, :])
                            qsrc.append(t)
                        for p in range(NP):
                            ps = pp.tile([128, S], F32, tag="psq", name="psq")
                            for k in range(KD):
                                wqt = wvp.tile([128, 128], BF16, tag="wqs", name="wqs")
                                nc.sync.dma_start(
                                    wqt[:], wq_[k * 128:(k + 1) * 128,
                                                p * 128:(p + 1) * 128])
                                nc.tensor.matmul(
                                    ps[:], wqt[:],
                                    qsrc[k][:], start=(k == 0),
                                    stop=(k == KD - 1))
                            nc.vector.tensor_scalar_add(q_dst[p][:], ps[:],
                                                        bq_[:, p:p + 1])
                    # V projection, token-major out into SBUF V tiles
                    for tt_ in range(TC // 128):
                        kt_ = tch * (TC // 128) + tt_
                        if lean_v:
                            psv = []
                            for vf in range(NOF):
                                pv = pp.tile([128, CH], F32, tag="psv",
                                             name="psv")
                                for k in range(KD):
                                    nc.tensor.matmul(
                                        pv[:],
                                        src[k][:, tt_ * 128:(tt_ + 1) * 128],
                                        wv_t[k][:, vf * CH:(vf + 1) * CH],
                                        start=(k == 0), stop=False)
                                psv.append(pv)
                        else:
                            psv = [pp.tile([128, CH], F32, tag=f"psv{vf}",
                                           name=f"psv{vf}") for vf in range(NOF)]
                            for k in range(KD):
                                for vf in range(NOF):
                                    nc.tensor.matmul(
                                        psv[vf][:],
                                        src[k][:, tt_ * 128:(tt_ + 1) * 128],
                                        wv_t[k][:, vf * CH:(vf + 1) * CH],
                                        start=(k == 0), stop=False)
                        for vf in range(NOF):
                            nc.tensor.matmul(psv[vf][:], ones128[:],
                                             bvrow[:, vf * CH:(vf + 1) * CH],
                                             start=False, stop=True)
                            nc.vector.tensor_copy(
                                vt_dst[kt_][:]
                                .rearrange("p (h c) -> p h c", c=HW)
                                [:, vf * HPC:(vf + 1) * HPC, 0:64],
                                psv[vf][:].rearrange("p (h c) -> p h c", c=64))
                        yield

        kvq_proj(yT_d, wq_d, wk_d, wv_d, bqT, bkT, bv_d, qfm, kfm1, None,
                 vt1, with_q=True)

        # ------------------------- attention -------------------------------
        def attention(*a, **kw):
            for _ in att_gen(*a, **kw):
                pass

        def att_gen(q_tiles, kfm_sb, kfm_dr, vt_src, out_tiles, causal,
                    ss_bufs=2, pav_bufs=2):
            with (
                tc.tile_pool(name="at_sb", bufs=3) as sp,
                tc.tile_pool(name="at_ps", bufs=ss_bufs, space="PSUM") as pp,
                tc.tile_pool(name="at_pav", bufs=pav_bufs, space="PSUM") as pav,
                tc.tile_pool(name="at_n", bufs=2) as npool,
            ):
                nkj = KC // 128

                def cw(kt):   # causal free width at key position kt
                    return 128 * (1 + (KT - 1 - kt) // NB) if causal else S

                for p in range(NP):
                    psA = pav.tile([128, S], F32, tag="psavA", name="psavA")
                    psB = pav.tile([128, S], F32, tag="psavB", name="psavB")
                    klt = None
                    pend = None   # skew-1: AV(kt-1) is emitted after scores(kt)

                    def emit_av(pr2_, kt_):
                        vl = vt_src[kt_]
                        w = cw(kt_)
                        nc.tensor.matmul(psA[0:65, 0:w],
                                         vl[:, 2 * p * HW:2 * p * HW + 65],
                                         pr2_[:, 0:w], start=(kt_ == 0),
                                         stop=(kt_ == KT - 1),
                                         skip_group_check=causal)
                        nc.tensor.matmul(psB[0:65, 0:w],
                                         vl[:, (2 * p + 1) * HW:(2 * p + 1) * HW + 65],
                                         pr2_[:, S:S + w], start=(kt_ == 0),
                                         stop=(kt_ == KT - 1),
                                         skip_group_check=causal)

                    for kt in range(KT):
                        w = cw(kt)
                        if kfm_sb is not None:
                            kl = kfm_sb[p]
                            ko = kt * 128
                        else:
                            kj = kt % nkj
                            if kj == 0:
                                ktc = kt // nkj
                                klt = sp.tile([128, KC], BF16, tag="kl", name="kl")
                                nc.sync.dma_start(
                                    klt[:], kfm_dr[p * 128:(p + 1) * 128,
                                                   ktc * KC:(ktc + 1) * KC])
                            kl = klt
                            ko = kj * 128
                        ss = pp.tile([128, 2 * S], F32, tag="ss", name="ss")
                        nc.tensor.matmul(
                            ss[:, 0:w], kl[0:64, ko:ko + 128],
                            q_tiles[p][0:64, 0:w], start=True, stop=True,
                            tile_position=(0, 0))
                        nc.tensor.matmul(
                            ss[:, S:S + w],
                            kl[64:128, ko:ko + 128],
                            q_tiles[p][64:128, 0:w], start=True, stop=True,
                            tile_position=(64, 0))
                        if pend is not None:
                            emit_av(*pend)
                        pr2 = sp.tile([128, 2 * S], BF16, tag="pr2", name="pr2")
                        bias_ap = kbias[:, kt:kt + 1] if causal else 0.0
                        nc.scalar.activation(
                            pr2[:].rearrange("p (h w) -> p h w", w=S)[:, :, 0:w],
                            ss[:].rearrange("p (h w) -> p h w", w=S)[:, :, 0:w],
                            AF.Exp, bias=bias_ap, scale=0.125)
                        if causal and (KT - 1 - kt) % NB == 0 \
                                and (KT - 1 - kt) // NB < TT:
                            # this position's last 128 columns are the
                            # diagonal slot's own tokens
                            for half in range(2):
                                nc.gpsimd.affine_select(
                                    out=pr2[:, half * S + w - 128:half * S + w],
                                    in_=pr2[:, half * S + w - 128:half * S + w],
                                    compare_op=ALU.is_ge, fill=0.0,
                                    base=0, channel_multiplier=-1,
                                    pattern=[[1, 128]])
                        pend = (pr2, kt)
                    emit_av(*pend)
                    recA = npool.tile([1, S], F32, tag="recA", name="recA")
                    recB = npool.tile([1, S], F32, tag="recB", name="recB")
                    nc.vector.reciprocal(recA[:], psA[64:65, :])
                    nc.vector.reciprocal(recB[:], psB[64:65, :])
                    rbA = npool.tile([64, S], F32, tag="rbA", name="rbA")
                    rbB = npool.tile([64, S], F32, tag="rbB", name="rbB")
                    nc.gpsimd.partition_broadcast(rbA[:], recA[:],
                                                  channels=64)
                    nc.gpsimd.partition_broadcast(rbB[:], recB[:],
                                                  channels=64)
                    avB = npool.tile([64, S], BF16, tag="avB", name="avB")
                    nc.vector.tensor_mul(out_tiles[p][0:64, :], psA[0:64, :],
                                         rbA[:])
                    nc.vector.tensor_mul(avB[:], psB[0:64, :], rbB[:])
                    nc.sync.dma_start(out_tiles[p][64:128, :], avB[:])
                    yield
                yield  # hold pools open until the driver drains us last

        # Interleave self-attention (ACT-heavy) with cross K/V projection
        # (PE-heavy, independent) so the PE fills attention's exp-wait gaps.
        g_att = att_gen(qfm, kfm1, None, vt1, avfm, causal=True,
                        ss_bufs=2, pav_bufs=1)
        g_cross = kvq_gen(xT_d, None, wk2_d, wv2_d, None, bk2T, bv2_d, None,
                          None, k2fm_s, vt2, with_q=False, src_bufs=1,
                          ps_bufs=1, lean_v=True)
        adv = 0
        next(g_att)             # opens att pools first (released last)
        while True:
            try:
                next(g_cross)
            except StopIteration:
                break
            if adv < NP:        # never exhaust g_att while g_cross is open
                next(g_att)
                adv += 1
        for _ in g_att:
            pass
        p_kv1.release()

        # ------------------ fc + residual + LN (token-major) ---------------
        def fc_ln(act_tiles, w_d_, brow_d_, resid, g_d_, b_d_, out_tiles, nk,
                  resid_is_dram=False):
            with (
                tc.tile_pool(name="fc_w", bufs=3) as wp,
                tc.tile_pool(name="fc_ps", bufs=1, space="PSUM") as pp,
                tc.tile_pool(name="fc_sb", bufs=1) as sp,
            ):
                brow_ = sp.tile([1, D], BF16, tag="brow", name="brow")
                nc.sync.dma_start(brow_[:], brow_d_[:])
                grow = sp.tile([1, D], F32, tag="grow", name="grow")
                brow2 = sp.tile([1, D], F32, tag="brow2", name="brow2")
                nc.sync.dma_start(grow[:], g_d_[:])
                nc.sync.dma_start(brow2[:], b_d_[:])
                g_ = sp.tile([128, D], F32, tag="gb_g", name="gb_g")
                b_ = sp.tile([128, D], F32, tag="gb_b", name="gb_b")
                nc.gpsimd.partition_broadcast(g_[:], grow[:], channels=128)
                nc.gpsimd.partition_broadcast(b_[:], brow2[:], channels=128)
                if resid_is_dram:
                    rtiles = []
                    for t in range(TT):
                        rt = sp.tile([128, D], F32, tag=f"res{t}", name=f"res{t}")
                        nc.sync.dma_start(rt[:], resid[t * 128:(t + 1) * 128, :])
                        rtiles.append(rt)
                    resid = rtiles
                ps = [[pp.tile([128, CH], F32, tag=f"fc{t}_{f}", name=f"fc{t}_{f}")
                       for f in range(NOF)] for t in range(TT)]
                for k in range(nk):
                    wt = wp.tile([128, D], BF16, tag="w", name="w")
                    nc.sync.dma_start(wt[:], w_d_[k * 128:(k + 1) * 128, :])
                    for t in range(TT):
                        for f in range(NOF):
                            nc.tensor.matmul(
                                ps[t][f][:],
                                act_tiles[k][:, t * 128:(t + 1) * 128],
                                wt[:, f * CH:(f + 1) * CH],
                                start=(k == 0), stop=False)
                for t in range(TT):
                    for f in range(NOF):
                        nc.tensor.matmul(ps[t][f][:], ones128[:],
                                         brow_[:, f * CH:(f + 1) * CH],
                                         start=False, stop=True)
                for t in range(TT):
                    r = sp.tile([128, D], F32, tag="r", name="r")
                    rs = [sp.tile([128, 1], F32, tag=f"rs{f}", name=f"rs{f}")
                          for f in range(NOF)]
                    for f in range(NOF):
                        nc.vector.scalar_tensor_tensor(
                            r[:, f * CH:(f + 1) * CH], ps[t][f][:], 1.0,
                            resid[t][:, f * CH:(f + 1) * CH],
                            op0=ALU.mult, op1=ALU.add, accum_out=rs[f][:])
                    rowsum = sp.tile([128, 1], F32, tag="rowsum", name="rowsum")
                    if NOF == 2:
                        nc.vector.tensor_add(rowsum[:], rs[0][:], rs[1][:])
                    else:
                        nc.vector.tensor_copy(rowsum[:], rs[0][:])
                    negmean = sp.tile([128, 1], F32, tag="negmean", name="negmean")
                    nc.scalar.mul(negmean[:], rowsum[:], -1.0 / D)
                    xnl = sp.tile([128, D], F32, tag="xnl", name="xnl")
                    nc.scalar.activation(xnl[:], r[:], AF.Identity,
                                         bias=negmean[:])
                    xsq = sp.tile([128, D], F32, tag="xsq", name="xsq")
                    ssq = sp.tile([128, 1], F32, tag="ssq", name="ssq")
                    nc.scalar.activation(xsq[:], xnl[:], AF.Square,
                                         accum_out=ssq[:])
                    sd = sp.tile([128, 1], F32, tag="sd", name="sd")
                    nc.scalar.activation(sd[:], ssq[:], AF.Sqrt,
                                         bias=eps_t[:], scale=1.0 / D)
                    rstd = sp.tile([128, 1], F32, tag="rstd", name="rstd")
                    nc.vector.reciprocal(rstd[:], sd[:])
                    tmp = sp.tile([128, D], F32, tag="tmp", name="tmp")
                    nc.vector.scalar_tensor_tensor(
                        tmp[:], xnl[:], rstd[:], g_[:], op0=ALU.mult,
                        op1=ALU.mult)
                    nc.vector.tensor_add(out_tiles[t][:], tmp[:], b_[:])

        fc_ln(avfm, wsa_d, bsa_d, yblk_d, gb_d["g1"], gb_d["b1"], y1, NP,
              resid_is_dram=True)

        p_right = tc.alloc_tile_pool(name="p_right", bufs=1, side="right")
        yT12 = [p_right.tile([128, S], BF16, tag=f"yT12_{k}", name=f"yT12_{k}") for k in range(KD)]
        y2 = [p_right.tile([128, D], F32, tag=f"y2_{t}", name=f"y2_{t}") for t in range(TT)]

        def transpose_to(src_tiles, dst_tiles):
            with (
                tc.tile_pool(name="tp_ps", bufs=2, space="PSUM") as pp,
                tc.tile_pool(name="tp_sb", bufs=2) as sp,
            ):
                for t in range(TT):
                    srcr = sp.tile([128, D], mybir.dt.float32r, tag="srcr", name="srcr")
                    nc.vector.tensor_copy(srcr[:], src_tiles[t][:])
                    for k in range(KD):
                        pst = pp.tile([128, 128], mybir.dt.float32r, tag="tp", name="tp")
                        nc.tensor.transpose(pst[:],
                                            srcr[:, k * 128:(k + 1) * 128],
                                            ident[:])
                        nc.vector.tensor_copy(
                            dst_tiles[k][:, t * 128:(t + 1) * 128], pst[:])

        transpose_to(y1, yT12)

        # ------------------------- cross attention -------------------------
        with (
            tc.tile_pool(name="q2_w", bufs=2) as wp,
            tc.tile_pool(name="q2_ps", bufs=2, space="PSUM") as pp,
        ):
            for p in range(NP):
                psq = pp.tile([128, S], F32, tag="psq2", name="psq2")
                for k in range(KD):
                    wt = wp.tile([128, 128], BF16, tag="wq2", name="wq2")
                    nc.sync.dma_start(
                        wt[:], wq2_d[k * 128:(k + 1) * 128,
                                     p * 128:(p + 1) * 128])
                    nc.tensor.matmul(psq[:], wt[:], yT12[k][:, 0:S],
                                     start=(k == 0), stop=(k == KD - 1))
                nc.vector.tensor_scalar_add(qfm[p][:], psq[:],
                                            bq2T[:, p:p + 1])

        attention(qfm, None, k2fm_s, vt2, avfm, causal=False)
        fc_ln(avfm, wca_d, bca_d, y1, gb_d["g2"], gb_d["b2"], y2, NP)
        transpose_to(y2, yT12)
        p_att.release()

        # ------------------------------ FFN ---------------------------------
        p_h = tc.alloc_tile_pool(name="p_h", bufs=1)
        hfm = [p_h.tile([128, S], BF16, tag=f"h{f}", name=f"h{f}") for f in range(FT)]
        with (
            tc.tile_pool(name="f1_w", bufs=4) as wp,
            tc.tile_pool(name="f1_ps", bufs=2, space="PSUM") as pp,
        ):
            for fg in range(FT // 4):
                psf = [pp.tile([128, S], F32, tag=f"psf{j}", name=f"psf{j}")
                       for j in range(4)]
                for k in range(KD):
                    wt = wp.tile([128, 512], BF16, tag="wff1", name="wff1")
                    nc.sync.dma_start(
                        wt[:], wff1_d[k * 128:(k + 1) * 128,
                                      fg * 512:(fg + 1) * 512])
                    for j in range(4):
                        nc.tensor.matmul(
                            psf[j][:], wt[:, j * 128:(j + 1) * 128],
                            yT12[k][:, 0:S], start=(k == 0),
                            stop=(k == KD - 1))
                for j in range(4):
                    f = fg * 4 + j
                    nc.scalar.activation(hfm[f][:], psf[j][:], AF.Relu,
                                         bias=bff1T[:, f:f + 1])
        out_f = [p_h.tile([128, D], F32, tag=f"out{t}", name=f"out{t}") for t in range(TT)]
        fc_ln(hfm, wff2_d, bff2_d, y2, gb_d["g3"], gb_d["b3"], out_f, FT)
        for t in range(TT):
            nc.sync.dma_start(out_d[t * 128:(t + 1) * 128, :], out_f[t][:])
        p_h.release()
        p_right.release()
        p_kv2.release()
        cpool.release()

    nc.compile()
    return nc


# ---------------------------------------------------------------- hosting ---
def core_qsel(cc, S, E):
    """Balanced causal assignment: slot j of core cc owns query tile
    KT-1-NB*j-cc. Returns the token indices (slot order)."""
    KT, NB = E // 128, E // S
    tiles = [KT - 1 - NB * j - cc for j in range(S // 128)]
    return np.concatenate([np.arange(t * 128, (t + 1) * 128) for t in tiles])


def make_inputs_for_core(full, b, o, D=1024, H=16, FF=4096, S=512, E=2048):
    HD = D // H
    KT = E // 128
    cc = o // S
    y = np.asarray(full["y"][b], dtype=np.float32)      # [E, D]
    x = np.asarray(full["x"][b], dtype=np.float32)
    qsel = core_qsel(cc, S, E)
    # key order: shifted natural, with cc dummy (bias-masked) tiles in front
    ykeys = np.concatenate([y[0:cc * 128], y[0:(KT - cc) * 128]], axis=0)
    yT = np.ascontiguousarray(ykeys.T)
    yTq = np.ascontiguousarray(y[qsel].T)
    xT = np.ascontiguousarray(x.T)
    kbias = np.zeros((128, KT), np.float32)
    kbias[:, 0:cc] = -30.0

    qkv_w = np.asarray(full["qkv_w"], np.float32).reshape(D, H, 3 * HD)
    wq = np.ascontiguousarray(qkv_w[:, :, 0:HD].reshape(D, D))
    wk = np.ascontiguousarray(qkv_w[:, :, HD:2 * HD].reshape(D, D))
    wv = np.ascontiguousarray(qkv_w[:, :, 2 * HD:].reshape(D, D))
    qkv_b = np.asarray(full["qkv_b"], np.float32).reshape(H, 3 * HD)
    bq = qkv_b[:, 0:HD].reshape(D)
    bk = qkv_b[:, HD:2 * HD].reshape(D)
    bv = qkv_b[:, 2 * HD:].reshape(D)
    kv_w = np.asarray(full["kv_w"], np.float32).reshape(D, H, 2 * HD)
    wk2 = np.ascontiguousarray(kv_w[:, :, 0:HD].reshape(D, D))
    wv2 = np.ascontiguousarray(kv_w[:, :, HD:].reshape(D, D))
    kv_b = np.asarray(full["kv_b"], np.float32).reshape(H, 2 * HD)
    bk2 = kv_b[:, 0:HD].reshape(D)
    bv2 = kv_b[:, HD:].reshape(D)

    def colT(v):   # [D] -> [128, D//128] (partition-major per 128-tile)
        return np.ascontiguousarray(v.reshape(-1, 128).T.astype(np.float32))

    def bf(a):
        return np.ascontiguousarray(np.asarray(a, np.float32)).astype(BF16NP)

    return {
        "yT": bf(yT), "yTq": bf(yTq),
        "yblk": np.ascontiguousarray(y[qsel]), "xT": bf(xT),
        "kbias": kbias, "ident": np.eye(128, dtype=np.float32),
        "ones128": bf(np.ones((1, 128), np.float32)),
        "oneshw": bf(np.concatenate([np.ones((128, H, 1), np.float32), np.zeros((128, H, 7), np.float32)], -1).reshape(128, H * 8)),
        "wq": bf(wq), "wk": bf(wk), "wv": bf(wv),
        "wsa": bf(full["sa_fc_w"]),
        "wq2": bf(full["q_w"]), "wk2": bf(wk2), "wv2": bf(wv2),
        "wca": bf(full["ca_fc_w"]),
        "wff1": bf(full["ff1_w"]),
        "wff2": bf(full["ff2_w"]),
        "bqT": colT(bq), "bkT": colT(bk),
        "bq2T": colT(np.asarray(full["q_b"], np.float32)), "bk2T": colT(bk2),
        "bff1T": colT(np.asarray(full["ff1_b"], np.float32)),
        "bv_r": bf(bv.reshape(1, D)), "bv2_r": bf(bv2.reshape(1, D)),
        "bsa_r": bf(np.asarray(full["sa_fc_b"], np.float32).reshape(1, D)),
        "bca_r": bf(np.asarray(full["ca_fc_b"], np.float32).reshape(1, D)),
        "bff2_r": bf(np.asarray(full["ff2_b"], np.float32).reshape(1, D)),
        "g1": np.asarray(full["g1"], np.float32).reshape(1, D),
        "b1": np.asarray(full["b1"], np.float32).reshape(1, D),
        "g2": np.asarray(full["g2"], np.float32).reshape(1, D),
        "b2": np.asarray(full["b2"], np.float32).reshape(1, D),
        "g3": np.asarray(full["g3"], np.float32).reshape(1, D),
        "b3": np.asarray(full["b3"], np.float32).reshape(1, D),
    }


# ------------------------------------------------------------------ runner --
def _install_neff_cache():
    from concourse import bass2jax
    if getattr(bass2jax, "_my_cache_installed", False):
        return
    os.makedirs(NEFF_CACHE, exist_ok=True)
    orig = bass2jax.compile_bir_kernel

    def cached(ant_bir_str, compile_dir_path, neff_name=None, **kw):
        key_bytes = ant_bir_str.encode() if isinstance(ant_bir_str, str) else ant_bir_str
        cpath = os.path.join(NEFF_CACHE,
                             hashlib.sha256(key_bytes).hexdigest() + ".neff")
        if os.path.exists(cpath):
            return cpath
        import shutil
        neff = orig(ant_bir_str, compile_dir_path, neff_name=neff_name, **kw)
        shutil.copy(neff, cpath)
        return cpath

    bass2jax.compile_bir_kernel = cached
    bass2jax._my_cache_installed = True


def run_spmd(nc, in_maps, n_cores, profile_dir=None):
    import jax
    from jax.sharding import Mesh, PartitionSpec
    from jax.experimental.shard_map import shard_map
    from concourse.bass2jax import (_bass_exec_p, partition_id_tensor,
                                    install_neuronx_cc_hook)
    _install_neff_cache()
    install_neuronx_cc_hook()

    partition_name = nc.partition_id_tensor.name if nc.partition_id_tensor else None
    in_names, out_names, out_avals, zero_outs = [], [], [], []
    for alloc in nc.m.functions[0].allocations:
        if not isinstance(alloc, mybir.MemoryLocationSet):
            continue
        name = alloc.memorylocations[0].name
        if alloc.kind == "ExternalInput":
            if name != partition_name:
                in_names.append(name)
        elif alloc.kind == "ExternalOutput":
            shape = tuple(alloc.tensor_shape)
            dtype = mybir.dt.np(alloc.dtype)
            out_names.append(name)
            out_avals.append(jax.core.ShapedArray(shape, dtype))
            zero_outs.append(np.zeros(shape, dtype))
    n_params = len(in_names)
    n_outs = len(out_avals)
    in_names.extend(out_names)
    if partition_name is not None:
        in_names.append(partition_name)
    donate = tuple(range(n_params, n_params + n_outs))

    def _body(*args):
        operands = list(args)
        if partition_name is not None:
            operands.append(partition_id_tensor())
        outs = _bass_exec_p.bind(
            *operands, out_avals=tuple(out_avals), in_names=tuple(in_names),
            out_names=tuple(out_names), lowering_input_output_aliases=(),
            sim_require_finite=True, sim_require_nnan=True, nc=nc)
        return tuple(outs)

    _body.__name__ = "u" + uuid.uuid4().hex[:12] + "_body"
    devices = jax.devices()[:n_cores]
    mesh = Mesh(np.asarray(devices), ("core",))
    sharded = jax.jit(
        shard_map(_body, mesh=mesh,
                  in_specs=(PartitionSpec("core"),) * (n_params + n_outs),
                  out_specs=(PartitionSpec("core"),) * n_outs,
                  check_rep=False),
        donate_argnums=donate, keep_unused=True)
    per_core = [[np.asarray(m[name]) for name in in_names[:n_params]]
                for m in in_maps]
    concat_in = [np.concatenate([per_core[c][i] for c in range(n_cores)], axis=0)
                 for i in range(n_params)]
    concat_zeros = [np.zeros((n_cores * z.shape[0], *z.shape[1:]), z.dtype)
                    for z in zero_outs]
    exec_ns = None
    if profile_dir is not None:
        from trn_agent_boot.trn_boot import _ntff_profile_via_ctypes
        if 'antenv.axon_hooks' not in sys.modules:
            mod = types.ModuleType('antenv.axon_hooks')
            _h = [None]
            mod.set_axon_ntff_profile_hook = lambda h: _h.__setitem__(0, h)
            mod.get_axon_ntff_profile_hook = lambda: _h[0]
            sys.modules['antenv.axon_hooks'] = mod
            import antenv
            antenv.axon_hooks = mod
        import antenv.axon_hooks as ah
        if ah.get_axon_ntff_profile_hook() is None:
            ah.set_axon_ntff_profile_hook(
                _ntff_profile_via_ctypes('/opt/axon/libaxon_pjrt.so'))
        hook = ah.get_axon_ntff_profile_hook()
        os.makedirs(profile_dir, exist_ok=True)
        compiled = sharded.lower(*concat_in, *concat_zeros).compile()
        with hook(profile_dir, [0]):
            out_arrs = compiled(*concat_in, *concat_zeros)
            out_arrs = [np.asarray(a) for a in out_arrs]
        exec_ns = _exec_time_from_ntff(profile_dir, nc)
    else:
        out_arrs = sharded(*concat_in, *concat_zeros)
        out_arrs = [np.asarray(a) for a in out_arrs]
    results = [
        {name: out_arrs[i].reshape(n_cores, *out_avals[i].shape)[c]
         for i, name in enumerate(out_names)}
        for c in range(n_cores)]
    return results, exec_ns


def _exec_time_from_ntff(profile_dir, nc):
    try:
        import gauge.profiler
        from concourse.bass_utils import _process_ntff_profile
        from concourse._compat import FishPath
        if not glob.glob(os.path.join(profile_dir, "*_body*.ntff")):
            return None
        profile = gauge.profiler.Profile(
            profile_path=FishPath(profile_dir), kernel_dev_mode=True,
            profile_on_exit=False, bass_kernel=nc.m, offline_processing=True,
            fname="*_body*", metadata={})
        r = _process_ntff_profile(profile, profile_dir, nc, [0], None, False,
                                  {}, False)
        return r.exec_time_ns
    except Exception:
        return None


_prog_cache = {}


def kernel(**inputs) -> np.ndarray:
    B, S_full, D = 2, 2048, 1024
    S, E = 512, 2048
    key = (D, S, E)
    if key not in _prog_cache:
        _prog_cache[key] = build_program(D=D, H=16, FF=4096, S=S, E=E,
                                         n_cores=8)
    nc = _prog_cache[key]
    in_maps = []
    for c in range(8):
        b, q = c // 4, c % 4
        in_maps.append(make_inputs_for_core(inputs, b, q * S))
    results, _ = run_spmd(nc, in_maps, 8)
    out = np.zeros((B, S_full, D), np.float32)
    for c in range(8):
        b, q = c // 4, c % 4
        out[b, core_qsel(q, S, E)] = results[c]["out_blk"]
    return out
